# revision 39
# baseline (speedup 1.0000x reference)
"""GCN node classifier (2x spmm + classifier + log_softmax) on 8 trn2 cores.

Strategy: destination-node 1D sharding with spmm linearity.
  spmm(A, x@W1 + b1) = (A x)@W1 + deg * b1^T      (deg = rowsum of A)
  spmm(A, h@W2 + b2)@Wc = (A h)@(W2 Wc) + deg * (b2 Wc)^T
so the gather tables are the RAW node features (x bf16 for layer 1,
relu-h bf16 for layer 2) — no dense pre-pass over all nodes, and the
layer weights are applied per dst tile after aggregation.

Each core owns 12,800 dst slots (100 tiles x 128 lanes). Host assigns
nodes to slots with a greedy 4-d balancer so that every (src-quarter,
dst-tile) edge bucket fits in KSEG=4 chunks of 128 edges (the int16
gather index forces 4 quarter views of the 102,400-row table). Per-edge
source rows are fetched with GPSIMD dma_gather (256B rows); the
segment-sum is a tensor-engine matmul against per-chunk scatter
matrices V[e, dst_lane] = edge_val[e] built on DVE with
(iota == ldst) * val, accumulated transposed (psT = Xg^T V) so the
per-tile epilogue can feed psT straight back as lhsT for the weight
matmul. log-softmax is fused per tile. Between layers the per-shard
relu-h table is AllGather'ed into a Shared DRAM tensor.
"""

import numpy as np
import ml_dtypes

from contextlib import ExitStack


# ---------------------------------------------------------------- config ---
class Cfg:
    M = 8                 # cores
    N_NODES = 100000
    N_EDGES = 1600000
    IN_DIM = 128
    HID = 64
    NCLS = 40
    NT = 100              # dst tiles per core (128 lanes each)
    KSEG = 4              # chunks (of 128 edges) per (quarter, tile) segment
    SLABC = 10            # chunks per gather slab
    SINGLE_PACKET = False  # multi-packet gathers (single-packet hangs >~1K idxs)
    NQUEUES = 4           # spread gathers over all 4 SWDGE queues
    MSGBUFS = 28
    IDXBUFS = 1
    GE = 5                # tiles per epilogue-matmul batch
    POOLV = 1             # of 16 chunks/tile, how many V-builds go to Pool
    PSB = 3               # psum accumulator ring depth
    VPB = 8               # V-tile ring depth
    LNG = 5               # tiles per deferred-Ln group
    NZBIAS = False        # set per-input: any of b1/b2/bc nonzero

    @property
    def PADSHARD(self):
        return self.NT * 128

    @property
    def NPAD(self):
        return self.PADSHARD * self.M

    @property
    def QROWS(self):
        return self.NPAD // 4

    @property
    def SEG(self):
        return self.KSEG * 128

    @property
    def CQ(self):
        return self.NT * self.KSEG          # chunks per quarter

    @property
    def NSLAB(self):
        assert self.CQ % self.SLABC == 0
        return self.CQ // self.SLABC        # gather slabs per quarter

    @property
    def CHUNKS(self):
        return 4 * self.CQ


CFG = Cfg()


# ------------------------------------------------------------- host plan ---
def _assign_slots(cfg, edge_row, edge_col):
    """Assign nodes to table slots so every (src-quarter, dst-tile) edge
    bucket holds <= KSEG*128 edges. Returns slot_of[node] -> [0, NPAD).

    Nodes are first split into 4 fixed quarter groups (so each node's
    src-quarter is pinned), then greedily packed into the 2*NT tiles of
    their own quarter balancing the 4-vector of per-src-quarter in-edge
    counts.
    """
    N, NPAD, QROWS, NT, M = cfg.N_NODES, cfg.NPAD, cfg.QROWS, cfg.NT, cfg.M
    TPQ = QROWS // 128                       # tiles per quarter (2 cores)
    rng = np.random.default_rng(12345)
    order = rng.permutation(N)
    qgrp = np.empty(N, dtype=np.int64)       # node -> quarter group
    npq = N // 4
    for q in range(4):
        qgrp[order[q * npq:(q + 1) * npq]] = q
    qgrp[order[4 * npq:]] = 3

    # per-node in-edge count by source quarter
    cnt = np.zeros((N, 4), dtype=np.int64)
    np.add.at(cnt, (edge_row, qgrp[edge_col]), 1)

    slot_of = np.empty(N, dtype=np.int64)
    for q in range(4):
        nodes = np.where(qgrp == q)[0]
        c = cnt[nodes].astype(np.float32)            # [nq, 4]
        tot = c.sum(1)
        o = np.argsort(-tot, kind="stable")
        nodes, c = nodes[o], c[o]
        loads = np.zeros((TPQ, 4), dtype=np.float32)
        fill = np.zeros(TPQ, dtype=np.int64)
        pos = np.empty(nodes.size, dtype=np.int64)
        for i in range(nodes.size):
            cand = np.max(loads + c[i], axis=1) + (fill >= 128) * 1e9
            b = int(np.argmin(cand))
            loads[b] += c[i]
            pos[i] = b * 128 + fill[b]
            fill[b] += 1
        slot_of[nodes] = q * QROWS + pos
    return slot_of


def _plan(cfg, edge_row, edge_col, edge_val, slot_of):
    """Bucket/sort/pad edges per core. Returns per-core arrays:
    idx16 [128, 4*CQ*128/16] int16, ldstT/valT [128, CHUNKS] bf16,
    plus degs [128, NT] f32 per core.
    """
    M, NT, KSEG, SEG, CQ, QROWS = cfg.M, cfg.NT, cfg.KSEG, cfg.SEG, cfg.CQ, cfg.QROWS
    PADSHARD = cfg.PADSHARD

    src_slot = slot_of[edge_col]
    dst_slot = slot_of[edge_row]
    q_of = src_slot // QROWS
    i_of = src_slot % QROWS
    core_of = dst_slot // PADSHARD
    dloc = dst_slot % PADSHARD
    t_of = dloc // 128
    l_of = dloc % 128

    deg = np.zeros(cfg.NPAD, dtype=np.float64)
    np.add.at(deg, dst_slot, edge_val.astype(np.float64))

    L = 4 * CQ * 128
    idx_all, ldst_all, val_all, deg_all = [], [], [], []
    for c in range(M):
        sel = core_of == c
        segid = q_of[sel] * NT + t_of[sel]
        order = np.argsort(segid, kind="stable")
        sid = segid[order]
        idx_s = i_of[sel][order]
        l_s = l_of[sel][order]
        v_s = edge_val[sel][order]

        counts = np.bincount(sid, minlength=4 * NT)
        if counts.max() > SEG:
            raise ValueError(f"segment overflow: {counts.max()} > {SEG}")
        starts = np.arange(4 * NT) * SEG
        pos = starts[sid] + (np.arange(sid.size)
                             - np.concatenate(([0], np.cumsum(counts)))[sid])

        idx = np.zeros(L, dtype=np.int16)
        ldst = np.zeros(L, dtype=np.float32)
        val = np.zeros(L, dtype=np.float32)
        idx[pos] = idx_s.astype(np.int16)
        ldst[pos] = l_s.astype(np.float32)
        val[pos] = v_s.astype(np.float32)

        # wrap indices: idx i -> [i%16, i//16], replicated on all 8 q7 cores
        idxw = np.tile(idx.reshape(-1, 16).T, (8, 1)).copy()
        ldstT = np.ascontiguousarray(ldst.reshape(-1, 128).T)
        valT = np.ascontiguousarray(val.reshape(-1, 128).T)
        degs = np.ascontiguousarray(
            deg[c * PADSHARD:(c + 1) * PADSHARD].reshape(NT, 128).T
        ).astype(np.float32)
        idx_all.append(idxw)
        ldst_all.append(ldstT)
        val_all.append(valT)
        deg_all.append(degs)
    return idx_all, ldst_all, val_all, deg_all


# --------------------------------------------------------- device program ---
def _build(cfg, timing=False, nzbias=False):
    import os
    from concourse import bacc, tile
    import concourse.mybir as mybir
    kdbg = bool(os.environ.get("KDBG"))

    f32 = mybir.dt.float32
    bf16 = mybir.dt.bfloat16
    i16 = mybir.dt.int16
    AOP = mybir.AluOpType
    ACT = mybir.ActivationFunctionType

    nc = bacc.Bacc("TRN2", target_bir_lowering=False, debug=False,
                   num_devices=1 if timing else cfg.M,
                   dynamic_dma_scratch_size=16384,
                   num_swdge_queues=cfg.NQUEUES)

    NPAD, QROWS, NT, KSEG, CQ, SLABC, NSLAB = (
        cfg.NPAD, cfg.QROWS, cfg.NT, cfg.KSEG, cfg.CQ, cfg.SLABC, cfg.NSLAB)
    CHUNKS, HID, NCLS, IN_DIM = cfg.CHUNKS, cfg.HID, cfg.NCLS, cfg.IN_DIM
    LQ16 = CQ * 128 // 16              # idx columns per quarter
    SLAB16 = SLABC * 128 // 16         # idx columns per slab

    # -------- I/O
    XG = nc.dram_tensor("xg", [NPAD, IN_DIM], bf16, kind="ExternalInput")
    IDX = nc.dram_tensor("idx", [128, 4 * LQ16], i16, kind="ExternalInput")
    LDST = nc.dram_tensor("ldst", [128, CHUNKS], f32, kind="ExternalInput")
    VAL = nc.dram_tensor("val", [128, CHUNKS], f32, kind="ExternalInput")
    DEG = nc.dram_tensor("deg", [128, NT], f32, kind="ExternalInput")
    W1 = nc.dram_tensor("w1", [IN_DIM, HID], bf16, kind="ExternalInput")
    W2C = nc.dram_tensor("w2c", [HID, NCLS], bf16, kind="ExternalInput")
    B1R = nc.dram_tensor("b1r", [128, HID], f32, kind="ExternalInput")
    BCOMBR = nc.dram_tensor("bcombr", [128, NCLS], f32, kind="ExternalInput")
    BCR = nc.dram_tensor("bcr", [128, NCLS], f32, kind="ExternalInput")
    IOTA = nc.dram_tensor("iota", [128, 128], bf16, kind="ExternalInput")
    f16 = mybir.dt.float16
    OUT = nc.dram_tensor("out", [cfg.PADSHARD, NCLS], f16, kind="ExternalOutput")
    HDBG = (nc.dram_tensor("hdbg", [cfg.PADSHARD, HID], bf16,
                           kind="ExternalOutput") if kdbg else None)
    LDBG = (nc.dram_tensor("ldbg", [128, NT * NCLS], f32,
                           kind="ExternalOutput") if kdbg else None)
    SMDBG = (nc.dram_tensor("smdbg", [128, NT], f32,
                            kind="ExternalOutput") if kdbg else None)

    # -------- internal DRAM
    HS = nc.dram_tensor("hshard", [cfg.PADSHARD, 128], bf16)    # cols 64+: junk
    HF = nc.dram_tensor("hfull", [NPAD, 128], bf16, addr_space="Shared")

    with tile.TileContext(nc) as tc, ExitStack() as top:
        # idx quarter 0 + V-build operands first: the first gathers and
        # V-builds depend only on these, so they issue before the consts.
        idxp = top.enter_context(tc.tile_pool(name="idxp", bufs=1))
        its = [[None] * 4 for _ in range(2)]
        its[0][0] = idxp.tile([128, LQ16], i16, tag="idx0_0", name="idx0_0")
        nc.sync.dma_start(out=its[0][0], in_=IDX[:, 0:LQ16])

        cpool = top.enter_context(tc.tile_pool(name="consts", bufs=1))
        iot = cpool.tile([128, 128], bf16)
        nc.sync.dma_start(out=iot, in_=IOTA[:, :])

        edg = top.enter_context(tc.tile_pool(name="edg", bufs=1))
        ldsts = edg.tile([128, CHUNKS], f32)
        nc.sync.dma_start(out=ldsts, in_=LDST[:, :])
        vals = edg.tile([128, CHUNKS], f32)
        nc.sync.dma_start(out=vals, in_=VAL[:, :])

        for q in range(1, 4):
            its[0][q] = idxp.tile([128, LQ16], i16, tag=f"idx0_{q}",
                                  name=f"idx0_{q}")
            nc.sync.dma_start(out=its[0][q],
                              in_=IDX[:, q * LQ16:(q + 1) * LQ16])

        w1s = cpool.tile([IN_DIM, HID], bf16)
        nc.sync.dma_start(out=w1s, in_=W1[:, :])
        w2cs = cpool.tile([HID, NCLS], bf16)
        nc.sync.dma_start(out=w2cs, in_=W2C[:, :])
        b1rs = cpool.tile([128, HID], f32)
        nc.sync.dma_start(out=b1rs, in_=B1R[:, :])
        bcombs = cpool.tile([128, NCLS], f32)
        nc.sync.dma_start(out=bcombs, in_=BCOMBR[:, :])
        bcrs = cpool.tile([128, NCLS], f32)
        nc.sync.dma_start(out=bcrs, in_=BCR[:, :])
        degs = cpool.tile([128, NT], f32)
        nc.sync.dma_start(out=degs, in_=DEG[:, :])

        for q in range(4):
            its[1][q] = idxp.tile([128, LQ16], i16, tag=f"idx1_{q}",
                                  name=f"idx1_{q}")
            nc.sync.dma_start(out=its[1][q],
                              in_=IDX[:, q * LQ16:(q + 1) * LQ16])

        # ============ spmm layer runner: per-tile single psum group across
        # all 4 quarters, accumulating transposed (psT = Xg^T V); epilogue
        # split into a per-tile part (cast) and a batched per-GE-tiles part
        # (weight matmuls etc) to keep the PE stream free of cross-engine
        # round trips.
        msg = top.enter_context(tc.tile_pool(name="msg", bufs=cfg.MSGBUFS))
        vp = top.enter_context(tc.tile_pool(name="vp", bufs=cfg.VPB))
        psb = top.enter_context(
            tc.tile_pool(name="psb", bufs=cfg.PSB, space="PSUM"))

        def spmm_layer(tab, width, lits, epi_tile, epi_group, ltag, gplan):
            gend = {}
            acc = 0
            for gsz in gplan:
                acc += gsz
                gend[acc - 1] = gsz
            assert acc == NT
            slabs = [[None] * NSLAB for _ in range(4)]

            def ensure_slab(q, s):
                if slabs[q][s] is None:
                    mt = msg.tile([128, SLABC, 128], bf16)
                    nc.gpsimd.dma_gather(
                        mt, tab[q * QROWS:(q + 1) * QROWS, :],
                        lits[q][:, s * SLAB16:(s + 1) * SLAB16],
                        num_idxs=SLABC * 128, num_idxs_reg=SLABC * 128,
                        elem_size=128, elem_step=128,
                        single_packet=cfg.SINGLE_PACKET,
                        queue_num=(q * NSLAB + s) % cfg.NQUEUES)
                    slabs[q][s] = mt
                return slabs[q][s]

            for t in range(NT):
                psfull = psb.tile([128, 128], f32, tag="ps", name="ps")
                ps = psfull if width == 128 else psfull[0:width, :]
                for q in range(4):
                    j0 = t * KSEG
                    vt = vp.tile([128, KSEG, 128], bf16)
                    for k in range(KSEG):
                        gj = q * CQ + j0 + k             # global chunk
                        veng = (nc.gpsimd
                                if q * KSEG + k >= 16 - cfg.POOLV
                                else nc.vector)
                        veng.tensor_scalar(
                            vt[:, k, :], iot, ldsts[:, gj:gj + 1],
                            vals[:, gj:gj + 1], AOP.is_equal, AOP.mult)
                    for k in range(KSEG):
                        j = j0 + k                       # chunk in quarter
                        mt = ensure_slab(q, j // SLABC)
                        nc.tensor.matmul(ps, lhsT=mt[:, j % SLABC, 0:width],
                                         rhs=vt[:, k, :],
                                         start=(q == 0 and k == 0),
                                         stop=(q == 3 and k == KSEG - 1))
                epi_tile(t, ps)
                if t in gend:
                    epi_group(t - gend[t] + 1, gend[t])

        # ================= layer 1: h = relu((A x)@W1 + deg*b1^T), store bf16
        with tc.tile_pool(name="tc1", bufs=cfg.GE + 2) as tp1, \
             tc.tile_pool(name="tc2", bufs=3) as tp2, \
             tc.tile_pool(name="pse", bufs=3, space="PSUM") as pse:
            pss1 = {}

            def epi1_tile(t, ps):
                pss = tp1.tile([IN_DIM, 128], bf16, tag="pss", name="pss")
                nc.scalar.activation(pss, ps, ACT.Copy)
                pss1[t] = pss

            def epi1_group(t0, n):
                for t in range(t0, t0 + n):
                    ph = pse.tile([128, HID], f32)
                    nc.tensor.matmul(ph, lhsT=pss1.pop(t), rhs=w1s,
                                     start=True, stop=True)
                    ht = tp2.tile([128, HID], bf16, tag="ht")
                    if nzbias:
                        tb = tp2.tile([128, HID], f32, tag="tb")
                        nc.vector.tensor_scalar(tb, b1rs, degs[:, t:t + 1],
                                                None, AOP.mult)
                        hsum = tp2.tile([128, HID], f32, tag="hsum")
                        nc.vector.tensor_tensor(hsum, ph, tb, AOP.add)
                        nc.scalar.activation(ht, hsum, ACT.Relu)
                    else:
                        nc.scalar.activation(ht, ph, ACT.Relu)
                    nc.sync.dma_start(out=HS[t * 128:(t + 1) * 128, 0:HID],
                                      in_=ht)

            spmm_layer(XG, IN_DIM, its[0], epi1_tile, epi1_group, "a",
                       [cfg.GE] * (NT // cfg.GE))
            if kdbg:
                nc.sync.dma_start(out=HDBG[:, :], in_=HS[:, 0:HID])
            if not timing:
                nc.gpsimd.collective_compute(
                    "AllGather", mybir.AluOpType.bypass,
                    replica_groups=[list(range(cfg.M))],
                    ins=[HS[:, :]], outs=[HF[:, :]])

        # ================= layer 2 + fused classifier/log_softmax
        with tc.tile_pool(name="te1", bufs=cfg.GE + 2) as te1, \
             tc.tile_pool(name="te2", bufs=3) as te2, \
             tc.tile_pool(name="te3", bufs=2) as te3, \
             tc.tile_pool(name="psf", bufs=3, space="PSUM") as psf:
            G = cfg.LNG
            gplan2 = [cfg.LNG] * (NT // cfg.LNG - 1) + [3, 2]
            gname = {}
            acc = 0
            for gi, gsz in enumerate(gplan2):
                for i in range(gsz):
                    gname[acc + i] = (gi, i, gsz)
                acc += gsz
            pss2 = {}
            state = {}

            def epi2_tile(t, ps):
                pss = te1.tile([HID, 128], bf16, tag="pss", name="pss")
                nc.scalar.activation(pss, ps, ACT.Copy)
                pss2[t] = pss

            def epi2_group(t0, n):
                for t in range(t0, t0 + n):
                    g, i, gsz = gname[t]
                    if i == 0:
                        state["lgg"] = te3.tile([128, G, NCLS], f32,
                                                tag="lgg", name="lgg")
                        state["negg"] = te3.tile([128, G], f32,
                                                 tag="negg", name="negg")
                        state["smg"] = te3.tile([128, G], f32,
                                                tag="smg", name="smg")
                    lgg, negg, smg = state["lgg"], state["negg"], state["smg"]
                    psl = psf.tile([128, NCLS], f32)
                    nc.tensor.matmul(psl, lhsT=pss2.pop(t), rhs=w2cs,
                                     start=True, stop=True)
                    if nzbias:
                        tb = te2.tile([128, NCLS], f32, tag="tb")
                        nc.vector.tensor_scalar(tb, bcombs, degs[:, t:t + 1],
                                                None, AOP.mult)
                        lg0 = te2.tile([128, NCLS], f32, tag="lg0")
                        nc.vector.tensor_tensor(lg0, psl, tb, AOP.add)
                        nc.gpsimd.tensor_tensor(lgg[:, i, :], lg0, bcrs,
                                                AOP.add)
                    else:
                        nc.scalar.activation(lgg[:, i, :], psl, ACT.Copy)
                    if i == gsz - 1:
                        if kdbg:
                            nc.sync.dma_start(
                                out=LDBG[:, g * G * NCLS:(g + 1) * G * NCLS],
                                in_=lgg.rearrange("p a b -> p (a b)"))
                            nc.sync.dma_start(out=SMDBG[:, g * G:(g + 1) * G],
                                              in_=smg)
                        nc.vector.tensor_reduce(negg[:, 0:gsz],
                                                lgg[:, 0:gsz, :],
                                                mybir.AxisListType.X, AOP.max,
                                                negate=True)
                        for ii in range(gsz):
                            et = te2.tile([128, NCLS], f32, tag="et")
                            nc.scalar.activation(et, lgg[:, ii, :], ACT.Exp,
                                                 bias=negg[:, ii:ii + 1],
                                                 accum_out=smg[:, ii:ii + 1])
                        lng = te2.tile([128, G], f32, tag="lng")
                        nc.scalar.activation(lng[:, 0:gsz], smg[:, 0:gsz],
                                             ACT.Ln)
                        shg = te2.tile([128, G], f32, tag="shg")
                        nc.vector.tensor_tensor(shg[:, 0:gsz], negg[:, 0:gsz],
                                                lng[:, 0:gsz], AOP.subtract)
                        for ii in range(gsz):
                            tt = t - gsz + 1 + ii
                            ot = te2.tile([128, NCLS], f16, tag="ot")
                            nc.vector.tensor_scalar(ot, lgg[:, ii, :],
                                                    shg[:, ii:ii + 1], None,
                                                    AOP.add)
                            nc.sync.dma_start(
                                out=OUT[tt * 128:(tt + 1) * 128, :], in_=ot)

            spmm_layer(HF, HID, its[1], epi2_tile, epi2_group, "b", gplan2)

    nc.compile()
    return nc


_NC_CACHE = {}


def _get_nc(cfg):
    key = (cfg.NT, cfg.KSEG, cfg.SLABC, cfg.NZBIAS)
    if key not in _NC_CACHE:
        _NC_CACHE[key] = _build(cfg, nzbias=cfg.NZBIAS)
    return _NC_CACHE[key]


# ------------------------------------------------------------------ main ---
def kernel(x, edge_row, edge_col, edge_val, W1, b1, W2, b2, Wc, bc,
           _run_kwargs=None):
    from concourse.bass_utils import run_bass_kernel_spmd

    cfg = CFG
    x = np.asarray(x, dtype=np.float32)
    edge_row = np.asarray(edge_row, dtype=np.int64)
    edge_col = np.asarray(edge_col, dtype=np.int64)
    edge_val = np.asarray(edge_val, dtype=np.float32)
    W1 = np.asarray(W1, dtype=np.float32)
    W2 = np.asarray(W2, dtype=np.float32)
    Wc = np.asarray(Wc, dtype=np.float32)
    b1 = np.asarray(b1, dtype=np.float32)
    b2 = np.asarray(b2, dtype=np.float32)
    bc = np.asarray(bc, dtype=np.float32)

    cfg.NZBIAS = bool(np.any(b1) or np.any(b2) or np.any(bc))
    slot_of = _assign_slots(cfg, edge_row, edge_col)
    try:
        idx_all, ldst_all, val_all, deg_all = _plan(
            cfg, edge_row, edge_col, edge_val, slot_of)
    except ValueError:
        cfg.KSEG += 1
        idx_all, ldst_all, val_all, deg_all = _plan(
            cfg, edge_row, edge_col, edge_val, slot_of)

    xg = np.zeros((cfg.NPAD, cfg.IN_DIM), dtype=ml_dtypes.bfloat16)
    xg[slot_of] = x.astype(ml_dtypes.bfloat16)

    w1h = W1.astype(ml_dtypes.bfloat16)
    w2c = (W2 @ Wc).astype(ml_dtypes.bfloat16)
    bcomb = b2 @ Wc
    iota = np.tile(np.arange(128, dtype=np.float32), (128, 1)).astype(
        ml_dtypes.bfloat16)
    b1r = np.tile(b1, (128, 1)).astype(np.float32)
    bcombr = np.tile(bcomb, (128, 1)).astype(np.float32)
    bcr = np.tile(bc, (128, 1)).astype(np.float32)

    nc = _get_nc(cfg)
    in_maps = []
    for c in range(cfg.M):
        in_maps.append({
            "xg": xg, "idx": idx_all[c], "ldst": ldst_all[c],
            "val": val_all[c], "deg": deg_all[c], "w1": w1h, "w2c": w2c,
            "b1r": b1r, "bcombr": bcombr, "bcr": bcr, "iota": iota,
        })
    kw = dict(_run_kwargs or {})
    res = run_bass_kernel_spmd(nc, in_maps, core_ids=list(range(cfg.M)), **kw)
    shard = np.concatenate(
        [res.results[c]["out"] for c in range(cfg.M)], axis=0)  # [NPAD, NCLS]
    out = shard[slot_of]
    kernel.last_results = res
    return out.astype(np.float32)


# revision 40
# speedup vs baseline: 1.0025x; 1.0025x over previous
"""GCN node classifier (2x spmm + classifier + log_softmax) on 8 trn2 cores.

Strategy: destination-node 1D sharding with spmm linearity.
  spmm(A, x@W1 + b1) = (A x)@W1 + deg * b1^T      (deg = rowsum of A)
  spmm(A, h@W2 + b2)@Wc = (A h)@(W2 Wc) + deg * (b2 Wc)^T
so the gather tables are the RAW node features (x bf16 for layer 1,
relu-h bf16 for layer 2) — no dense pre-pass over all nodes, and the
layer weights are applied per dst tile after aggregation.

Each core owns 12,800 dst slots (100 tiles x 128 lanes). Host assigns
nodes to slots with a greedy 4-d balancer so that every (src-quarter,
dst-tile) edge bucket fits in KSEG=4 chunks of 128 edges (the int16
gather index forces 4 quarter views of the 102,400-row table). Per-edge
source rows are fetched with GPSIMD dma_gather (256B rows); the
segment-sum is a tensor-engine matmul against per-chunk scatter
matrices V[e, dst_lane] = edge_val[e] built on DVE with
(iota == ldst) * val, accumulated transposed (psT = Xg^T V) so the
per-tile epilogue can feed psT straight back as lhsT for the weight
matmul. log-softmax is fused per tile. Between layers the per-shard
relu-h table is AllGather'ed into a Shared DRAM tensor.
"""

import numpy as np
import ml_dtypes

from contextlib import ExitStack


# ---------------------------------------------------------------- config ---
class Cfg:
    M = 8                 # cores
    N_NODES = 100000
    N_EDGES = 1600000
    IN_DIM = 128
    HID = 64
    NCLS = 40
    NT = 99               # dst tiles per core (128 lanes each)
    KSEG = 4              # chunks (of 128 edges) per (quarter, tile) segment
    SLABC = 11            # chunks per gather slab
    SINGLE_PACKET = False  # multi-packet gathers (single-packet hangs >~1K idxs)
    NQUEUES = 4           # spread gathers over all 4 SWDGE queues
    MSGBUFS = 25
    IDXBUFS = 1
    GE = 5                # tiles per epilogue-matmul batch
    POOLV = 1             # of 16 chunks/tile, how many V-builds go to Pool
    PSB = 3               # psum accumulator ring depth
    VPB = 8               # V-tile ring depth
    LNG = 5               # tiles per deferred-Ln group
    NZBIAS = False        # set per-input: any of b1/b2/bc nonzero

    @property
    def PADSHARD(self):
        return self.NT * 128

    @property
    def NPAD(self):
        return self.PADSHARD * self.M

    @property
    def QROWS(self):
        return self.NPAD // 4

    @property
    def SEG(self):
        return self.KSEG * 128

    @property
    def CQ(self):
        return self.NT * self.KSEG          # chunks per quarter

    @property
    def NSLAB(self):
        assert self.CQ % self.SLABC == 0
        return self.CQ // self.SLABC        # gather slabs per quarter

    @property
    def CHUNKS(self):
        return 4 * self.CQ


CFG = Cfg()


# ------------------------------------------------------------- host plan ---
def _assign_slots(cfg, edge_row, edge_col):
    """Assign nodes to table slots so every (src-quarter, dst-tile) edge
    bucket holds <= KSEG*128 edges. Returns slot_of[node] -> [0, NPAD).

    Nodes are first split into 4 fixed quarter groups (so each node's
    src-quarter is pinned), then greedily packed into the 2*NT tiles of
    their own quarter balancing the 4-vector of per-src-quarter in-edge
    counts.
    """
    N, NPAD, QROWS, NT, M = cfg.N_NODES, cfg.NPAD, cfg.QROWS, cfg.NT, cfg.M
    TPQ = QROWS // 128                       # tiles per quarter (2 cores)
    rng = np.random.default_rng(12345)
    order = rng.permutation(N)
    qgrp = np.empty(N, dtype=np.int64)       # node -> quarter group
    npq = N // 4
    for q in range(4):
        qgrp[order[q * npq:(q + 1) * npq]] = q
    qgrp[order[4 * npq:]] = 3

    # per-node in-edge count by source quarter
    cnt = np.zeros((N, 4), dtype=np.int64)
    np.add.at(cnt, (edge_row, qgrp[edge_col]), 1)

    slot_of = np.empty(N, dtype=np.int64)
    for q in range(4):
        nodes = np.where(qgrp == q)[0]
        c = cnt[nodes].astype(np.float32)            # [nq, 4]
        tot = c.sum(1)
        o = np.argsort(-tot, kind="stable")
        nodes, c = nodes[o], c[o]
        loads = np.zeros((TPQ, 4), dtype=np.float32)
        fill = np.zeros(TPQ, dtype=np.int64)
        pos = np.empty(nodes.size, dtype=np.int64)
        for i in range(nodes.size):
            cand = np.max(loads + c[i], axis=1) + (fill >= 128) * 1e9
            b = int(np.argmin(cand))
            loads[b] += c[i]
            pos[i] = b * 128 + fill[b]
            fill[b] += 1
        slot_of[nodes] = q * QROWS + pos
    return slot_of


def _plan(cfg, edge_row, edge_col, edge_val, slot_of):
    """Bucket/sort/pad edges per core. Returns per-core arrays:
    idx16 [128, 4*CQ*128/16] int16, ldstT/valT [128, CHUNKS] bf16,
    plus degs [128, NT] f32 per core.
    """
    M, NT, KSEG, SEG, CQ, QROWS = cfg.M, cfg.NT, cfg.KSEG, cfg.SEG, cfg.CQ, cfg.QROWS
    PADSHARD = cfg.PADSHARD

    src_slot = slot_of[edge_col]
    dst_slot = slot_of[edge_row]
    q_of = src_slot // QROWS
    i_of = src_slot % QROWS
    core_of = dst_slot // PADSHARD
    dloc = dst_slot % PADSHARD
    t_of = dloc // 128
    l_of = dloc % 128

    deg = np.zeros(cfg.NPAD, dtype=np.float64)
    np.add.at(deg, dst_slot, edge_val.astype(np.float64))

    L = 4 * CQ * 128
    idx_all, ldst_all, val_all, deg_all = [], [], [], []
    for c in range(M):
        sel = core_of == c
        segid = q_of[sel] * NT + t_of[sel]
        order = np.argsort(segid, kind="stable")
        sid = segid[order]
        idx_s = i_of[sel][order]
        l_s = l_of[sel][order]
        v_s = edge_val[sel][order]

        counts = np.bincount(sid, minlength=4 * NT)
        if counts.max() > SEG:
            raise ValueError(f"segment overflow: {counts.max()} > {SEG}")
        starts = np.arange(4 * NT) * SEG
        pos = starts[sid] + (np.arange(sid.size)
                             - np.concatenate(([0], np.cumsum(counts)))[sid])

        idx = np.zeros(L, dtype=np.int16)
        ldst = np.zeros(L, dtype=np.float32)
        val = np.zeros(L, dtype=np.float32)
        idx[pos] = idx_s.astype(np.int16)
        ldst[pos] = l_s.astype(np.float32)
        val[pos] = v_s.astype(np.float32)

        # wrap indices: idx i -> [i%16, i//16], replicated on all 8 q7 cores
        idxw = np.tile(idx.reshape(-1, 16).T, (8, 1)).copy()
        ldstT = np.ascontiguousarray(ldst.reshape(-1, 128).T)
        valT = np.ascontiguousarray(val.reshape(-1, 128).T)
        degs = np.ascontiguousarray(
            deg[c * PADSHARD:(c + 1) * PADSHARD].reshape(NT, 128).T
        ).astype(np.float32)
        idx_all.append(idxw)
        ldst_all.append(ldstT)
        val_all.append(valT)
        deg_all.append(degs)
    return idx_all, ldst_all, val_all, deg_all


# --------------------------------------------------------- device program ---
def _build(cfg, timing=False, nzbias=False):
    import os
    from concourse import bacc, tile
    import concourse.mybir as mybir
    kdbg = bool(os.environ.get("KDBG"))

    f32 = mybir.dt.float32
    bf16 = mybir.dt.bfloat16
    i16 = mybir.dt.int16
    AOP = mybir.AluOpType
    ACT = mybir.ActivationFunctionType

    nc = bacc.Bacc("TRN2", target_bir_lowering=False, debug=False,
                   num_devices=1 if timing else cfg.M,
                   dynamic_dma_scratch_size=16384,
                   num_swdge_queues=cfg.NQUEUES)

    NPAD, QROWS, NT, KSEG, CQ, SLABC, NSLAB = (
        cfg.NPAD, cfg.QROWS, cfg.NT, cfg.KSEG, cfg.CQ, cfg.SLABC, cfg.NSLAB)
    CHUNKS, HID, NCLS, IN_DIM = cfg.CHUNKS, cfg.HID, cfg.NCLS, cfg.IN_DIM
    LQ16 = CQ * 128 // 16              # idx columns per quarter
    SLAB16 = SLABC * 128 // 16         # idx columns per slab

    # -------- I/O
    XG = nc.dram_tensor("xg", [NPAD, IN_DIM], bf16, kind="ExternalInput")
    IDX = nc.dram_tensor("idx", [128, 4 * LQ16], i16, kind="ExternalInput")
    LDST = nc.dram_tensor("ldst", [128, CHUNKS], f32, kind="ExternalInput")
    VAL = nc.dram_tensor("val", [128, CHUNKS], f32, kind="ExternalInput")
    DEG = nc.dram_tensor("deg", [128, NT], f32, kind="ExternalInput")
    W1 = nc.dram_tensor("w1", [IN_DIM, HID], bf16, kind="ExternalInput")
    W2C = nc.dram_tensor("w2c", [HID, NCLS], bf16, kind="ExternalInput")
    B1R = nc.dram_tensor("b1r", [128, HID], f32, kind="ExternalInput")
    BCOMBR = nc.dram_tensor("bcombr", [128, NCLS], f32, kind="ExternalInput")
    BCR = nc.dram_tensor("bcr", [128, NCLS], f32, kind="ExternalInput")
    IOTA = nc.dram_tensor("iota", [128, 128], bf16, kind="ExternalInput")
    f16 = mybir.dt.float16
    OUT = nc.dram_tensor("out", [cfg.PADSHARD, NCLS], f16, kind="ExternalOutput")
    HDBG = (nc.dram_tensor("hdbg", [cfg.PADSHARD, HID], bf16,
                           kind="ExternalOutput") if kdbg else None)
    LDBG = (nc.dram_tensor("ldbg", [128, NT * NCLS], f32,
                           kind="ExternalOutput") if kdbg else None)
    SMDBG = (nc.dram_tensor("smdbg", [128, NT], f32,
                            kind="ExternalOutput") if kdbg else None)

    # -------- internal DRAM
    HS = nc.dram_tensor("hshard", [cfg.PADSHARD, 128], bf16)    # cols 64+: junk
    HF = nc.dram_tensor("hfull", [NPAD, 128], bf16, addr_space="Shared")

    with tile.TileContext(nc) as tc, ExitStack() as top:
        # idx quarter 0 + V-build operands first: the first gathers and
        # V-builds depend only on these, so they issue before the consts.
        idxp = top.enter_context(tc.tile_pool(name="idxp", bufs=1))
        its = [[None] * 4 for _ in range(2)]
        its[0][0] = idxp.tile([128, LQ16], i16, tag="idx0_0", name="idx0_0")
        nc.sync.dma_start(out=its[0][0], in_=IDX[:, 0:LQ16])

        cpool = top.enter_context(tc.tile_pool(name="consts", bufs=1))
        iot = cpool.tile([128, 128], bf16)
        nc.sync.dma_start(out=iot, in_=IOTA[:, :])

        edg = top.enter_context(tc.tile_pool(name="edg", bufs=1))
        ldsts = edg.tile([128, CHUNKS], f32)
        nc.sync.dma_start(out=ldsts, in_=LDST[:, :])
        vals = edg.tile([128, CHUNKS], f32)
        nc.sync.dma_start(out=vals, in_=VAL[:, :])

        for q in range(1, 4):
            its[0][q] = idxp.tile([128, LQ16], i16, tag=f"idx0_{q}",
                                  name=f"idx0_{q}")
            nc.sync.dma_start(out=its[0][q],
                              in_=IDX[:, q * LQ16:(q + 1) * LQ16])

        w1s = cpool.tile([IN_DIM, HID], bf16)
        nc.sync.dma_start(out=w1s, in_=W1[:, :])
        w2cs = cpool.tile([HID, NCLS], bf16)
        nc.sync.dma_start(out=w2cs, in_=W2C[:, :])
        b1rs = cpool.tile([128, HID], f32)
        nc.sync.dma_start(out=b1rs, in_=B1R[:, :])
        bcombs = cpool.tile([128, NCLS], f32)
        nc.sync.dma_start(out=bcombs, in_=BCOMBR[:, :])
        bcrs = cpool.tile([128, NCLS], f32)
        nc.sync.dma_start(out=bcrs, in_=BCR[:, :])
        degs = cpool.tile([128, NT], f32)
        nc.sync.dma_start(out=degs, in_=DEG[:, :])

        for q in range(4):
            its[1][q] = idxp.tile([128, LQ16], i16, tag=f"idx1_{q}",
                                  name=f"idx1_{q}")
            nc.sync.dma_start(out=its[1][q],
                              in_=IDX[:, q * LQ16:(q + 1) * LQ16])

        # ============ spmm layer runner: per-tile single psum group across
        # all 4 quarters, accumulating transposed (psT = Xg^T V); epilogue
        # split into a per-tile part (cast) and a batched per-GE-tiles part
        # (weight matmuls etc) to keep the PE stream free of cross-engine
        # round trips.
        msg = top.enter_context(tc.tile_pool(name="msg", bufs=cfg.MSGBUFS))
        vp = top.enter_context(tc.tile_pool(name="vp", bufs=cfg.VPB))
        psb = top.enter_context(
            tc.tile_pool(name="psb", bufs=cfg.PSB, space="PSUM"))

        def spmm_layer(tab, width, lits, epi_tile, epi_group, ltag, gplan):
            gend = {}
            acc = 0
            for gsz in gplan:
                acc += gsz
                gend[acc - 1] = gsz
            assert acc == NT
            slabs = [[None] * NSLAB for _ in range(4)]

            def ensure_slab(q, s):
                if slabs[q][s] is None:
                    mt = msg.tile([128, SLABC, 128], bf16)
                    nc.gpsimd.dma_gather(
                        mt, tab[q * QROWS:(q + 1) * QROWS, :],
                        lits[q][:, s * SLAB16:(s + 1) * SLAB16],
                        num_idxs=SLABC * 128, num_idxs_reg=SLABC * 128,
                        elem_size=128, elem_step=128,
                        single_packet=cfg.SINGLE_PACKET,
                        queue_num=(q * NSLAB + s) % cfg.NQUEUES)
                    slabs[q][s] = mt
                return slabs[q][s]

            for t in range(NT):
                psfull = psb.tile([128, 128], f32, tag="ps", name="ps")
                ps = psfull if width == 128 else psfull[0:width, :]
                for q in range(4):
                    j0 = t * KSEG
                    vt = vp.tile([128, KSEG, 128], bf16)
                    for k in range(KSEG):
                        gj = q * CQ + j0 + k             # global chunk
                        veng = (nc.gpsimd
                                if q * KSEG + k >= 16 - cfg.POOLV
                                else nc.vector)
                        veng.tensor_scalar(
                            vt[:, k, :], iot, ldsts[:, gj:gj + 1],
                            vals[:, gj:gj + 1], AOP.is_equal, AOP.mult)
                    for k in range(KSEG):
                        j = j0 + k                       # chunk in quarter
                        mt = ensure_slab(q, j // SLABC)
                        nc.tensor.matmul(ps, lhsT=mt[:, j % SLABC, 0:width],
                                         rhs=vt[:, k, :],
                                         start=(q == 0 and k == 0),
                                         stop=(q == 3 and k == KSEG - 1))
                epi_tile(t, ps)
                if t in gend:
                    epi_group(t - gend[t] + 1, gend[t])

        # ================= layer 1: h = relu((A x)@W1 + deg*b1^T), store bf16
        with tc.tile_pool(name="tc1", bufs=cfg.GE + 2) as tp1, \
             tc.tile_pool(name="tc2", bufs=3) as tp2, \
             tc.tile_pool(name="pse", bufs=3, space="PSUM") as pse:
            pss1 = {}

            def epi1_tile(t, ps):
                pss = tp1.tile([IN_DIM, 128], bf16, tag="pss", name="pss")
                nc.scalar.activation(pss, ps, ACT.Copy)
                pss1[t] = pss

            def epi1_group(t0, n):
                for t in range(t0, t0 + n):
                    ph = pse.tile([128, HID], f32)
                    nc.tensor.matmul(ph, lhsT=pss1.pop(t), rhs=w1s,
                                     start=True, stop=True)
                    ht = tp2.tile([128, HID], bf16, tag="ht")
                    if nzbias:
                        tb = tp2.tile([128, HID], f32, tag="tb")
                        nc.vector.tensor_scalar(tb, b1rs, degs[:, t:t + 1],
                                                None, AOP.mult)
                        hsum = tp2.tile([128, HID], f32, tag="hsum")
                        nc.vector.tensor_tensor(hsum, ph, tb, AOP.add)
                        nc.scalar.activation(ht, hsum, ACT.Relu)
                    else:
                        nc.scalar.activation(ht, ph, ACT.Relu)
                    nc.sync.dma_start(out=HS[t * 128:(t + 1) * 128, 0:HID],
                                      in_=ht)

            spmm_layer(XG, IN_DIM, its[0], epi1_tile, epi1_group, "a",
                       [cfg.GE] * (NT // cfg.GE)
                       + ([NT % cfg.GE] if NT % cfg.GE else []))
            if kdbg:
                nc.sync.dma_start(out=HDBG[:, :], in_=HS[:, 0:HID])
            if not timing:
                nc.gpsimd.collective_compute(
                    "AllGather", mybir.AluOpType.bypass,
                    replica_groups=[list(range(cfg.M))],
                    ins=[HS[:, :]], outs=[HF[:, :]])

        # ================= layer 2 + fused classifier/log_softmax
        with tc.tile_pool(name="te1", bufs=cfg.GE + 2) as te1, \
             tc.tile_pool(name="te2", bufs=3) as te2, \
             tc.tile_pool(name="te3", bufs=2) as te3, \
             tc.tile_pool(name="psf", bufs=3, space="PSUM") as psf:
            G = cfg.LNG
            _rem = NT - cfg.LNG * (NT // cfg.LNG - 1)
            gplan2 = [cfg.LNG] * (NT // cfg.LNG - 1) + [_rem - 5, 3, 2]
            gname = {}
            acc = 0
            for gi, gsz in enumerate(gplan2):
                for i in range(gsz):
                    gname[acc + i] = (gi, i, gsz)
                acc += gsz
            pss2 = {}
            state = {}

            def epi2_tile(t, ps):
                pss = te1.tile([HID, 128], bf16, tag="pss", name="pss")
                nc.scalar.activation(pss, ps, ACT.Copy)
                pss2[t] = pss

            def epi2_group(t0, n):
                for t in range(t0, t0 + n):
                    g, i, gsz = gname[t]
                    if i == 0:
                        state["lgg"] = te3.tile([128, G, NCLS], f32,
                                                tag="lgg", name="lgg")
                        state["negg"] = te3.tile([128, G], f32,
                                                 tag="negg", name="negg")
                        state["smg"] = te3.tile([128, G], f32,
                                                tag="smg", name="smg")
                    lgg, negg, smg = state["lgg"], state["negg"], state["smg"]
                    psl = psf.tile([128, NCLS], f32)
                    nc.tensor.matmul(psl, lhsT=pss2.pop(t), rhs=w2cs,
                                     start=True, stop=True)
                    if nzbias:
                        tb = te2.tile([128, NCLS], f32, tag="tb")
                        nc.vector.tensor_scalar(tb, bcombs, degs[:, t:t + 1],
                                                None, AOP.mult)
                        lg0 = te2.tile([128, NCLS], f32, tag="lg0")
                        nc.vector.tensor_tensor(lg0, psl, tb, AOP.add)
                        nc.gpsimd.tensor_tensor(lgg[:, i, :], lg0, bcrs,
                                                AOP.add)
                    else:
                        nc.scalar.activation(lgg[:, i, :], psl, ACT.Copy)
                    if i == gsz - 1:
                        if kdbg:
                            nc.sync.dma_start(
                                out=LDBG[:, g * G * NCLS:(g + 1) * G * NCLS],
                                in_=lgg.rearrange("p a b -> p (a b)"))
                            nc.sync.dma_start(out=SMDBG[:, g * G:(g + 1) * G],
                                              in_=smg)
                        nc.vector.tensor_reduce(negg[:, 0:gsz],
                                                lgg[:, 0:gsz, :],
                                                mybir.AxisListType.X, AOP.max,
                                                negate=True)
                        for ii in range(gsz):
                            et = te2.tile([128, NCLS], f32, tag="et")
                            nc.scalar.activation(et, lgg[:, ii, :], ACT.Exp,
                                                 bias=negg[:, ii:ii + 1],
                                                 accum_out=smg[:, ii:ii + 1])
                        lng = te2.tile([128, G], f32, tag="lng")
                        nc.scalar.activation(lng[:, 0:gsz], smg[:, 0:gsz],
                                             ACT.Ln)
                        shg = te2.tile([128, G], f32, tag="shg")
                        nc.vector.tensor_tensor(shg[:, 0:gsz], negg[:, 0:gsz],
                                                lng[:, 0:gsz], AOP.subtract)
                        for ii in range(gsz):
                            tt = t - gsz + 1 + ii
                            ot = te2.tile([128, NCLS], f16, tag="ot")
                            nc.vector.tensor_scalar(ot, lgg[:, ii, :],
                                                    shg[:, ii:ii + 1], None,
                                                    AOP.add)
                            nc.sync.dma_start(
                                out=OUT[tt * 128:(tt + 1) * 128, :], in_=ot)

            spmm_layer(HF, HID, its[1], epi2_tile, epi2_group, "b", gplan2)

    nc.compile()
    return nc


_NC_CACHE = {}


def _get_nc(cfg):
    key = (cfg.NT, cfg.KSEG, cfg.SLABC, cfg.NZBIAS)
    if key not in _NC_CACHE:
        _NC_CACHE[key] = _build(cfg, nzbias=cfg.NZBIAS)
    return _NC_CACHE[key]


# ------------------------------------------------------------------ main ---
def kernel(x, edge_row, edge_col, edge_val, W1, b1, W2, b2, Wc, bc,
           _run_kwargs=None):
    from concourse.bass_utils import run_bass_kernel_spmd

    cfg = CFG
    x = np.asarray(x, dtype=np.float32)
    edge_row = np.asarray(edge_row, dtype=np.int64)
    edge_col = np.asarray(edge_col, dtype=np.int64)
    edge_val = np.asarray(edge_val, dtype=np.float32)
    W1 = np.asarray(W1, dtype=np.float32)
    W2 = np.asarray(W2, dtype=np.float32)
    Wc = np.asarray(Wc, dtype=np.float32)
    b1 = np.asarray(b1, dtype=np.float32)
    b2 = np.asarray(b2, dtype=np.float32)
    bc = np.asarray(bc, dtype=np.float32)

    cfg.NZBIAS = bool(np.any(b1) or np.any(b2) or np.any(bc))
    slot_of = _assign_slots(cfg, edge_row, edge_col)
    try:
        idx_all, ldst_all, val_all, deg_all = _plan(
            cfg, edge_row, edge_col, edge_val, slot_of)
    except ValueError:
        cfg.KSEG += 1
        idx_all, ldst_all, val_all, deg_all = _plan(
            cfg, edge_row, edge_col, edge_val, slot_of)

    xg = np.zeros((cfg.NPAD, cfg.IN_DIM), dtype=ml_dtypes.bfloat16)
    xg[slot_of] = x.astype(ml_dtypes.bfloat16)

    w1h = W1.astype(ml_dtypes.bfloat16)
    w2c = (W2 @ Wc).astype(ml_dtypes.bfloat16)
    bcomb = b2 @ Wc
    iota = np.tile(np.arange(128, dtype=np.float32), (128, 1)).astype(
        ml_dtypes.bfloat16)
    b1r = np.tile(b1, (128, 1)).astype(np.float32)
    bcombr = np.tile(bcomb, (128, 1)).astype(np.float32)
    bcr = np.tile(bc, (128, 1)).astype(np.float32)

    nc = _get_nc(cfg)
    in_maps = []
    for c in range(cfg.M):
        in_maps.append({
            "xg": xg, "idx": idx_all[c], "ldst": ldst_all[c],
            "val": val_all[c], "deg": deg_all[c], "w1": w1h, "w2c": w2c,
            "b1r": b1r, "bcombr": bcombr, "bcr": bcr, "iota": iota,
        })
    kw = dict(_run_kwargs or {})
    res = run_bass_kernel_spmd(nc, in_maps, core_ids=list(range(cfg.M)), **kw)
    shard = np.concatenate(
        [res.results[c]["out"] for c in range(cfg.M)], axis=0)  # [NPAD, NCLS]
    out = shard[slot_of]
    kernel.last_results = res
    return out.astype(np.float32)


# revision 41
# speedup vs baseline: 1.0097x; 1.0071x over previous
"""GCN node classifier (2x spmm + classifier + log_softmax) on 8 trn2 cores.

Strategy: destination-node 1D sharding with spmm linearity.
  spmm(A, x@W1 + b1) = (A x)@W1 + deg * b1^T      (deg = rowsum of A)
  spmm(A, h@W2 + b2)@Wc = (A h)@(W2 Wc) + deg * (b2 Wc)^T
so the gather tables are the RAW node features (x bf16 for layer 1,
relu-h bf16 for layer 2) — no dense pre-pass over all nodes, and the
layer weights are applied per dst tile after aggregation.

Each core owns 12,800 dst slots (100 tiles x 128 lanes). Host assigns
nodes to slots with a greedy 4-d balancer so that every (src-quarter,
dst-tile) edge bucket fits in KSEG=4 chunks of 128 edges (the int16
gather index forces 4 quarter views of the 102,400-row table). Per-edge
source rows are fetched with GPSIMD dma_gather (256B rows); the
segment-sum is a tensor-engine matmul against per-chunk scatter
matrices V[e, dst_lane] = edge_val[e] built on DVE with
(iota == ldst) * val, accumulated transposed (psT = Xg^T V) so the
per-tile epilogue can feed psT straight back as lhsT for the weight
matmul. log-softmax is fused per tile. Between layers the per-shard
relu-h table is AllGather'ed into a Shared DRAM tensor.
"""

import numpy as np
import ml_dtypes

from contextlib import ExitStack


# ---------------------------------------------------------------- config ---
class Cfg:
    M = 8                 # cores
    N_NODES = 100000
    N_EDGES = 1600000
    IN_DIM = 128
    HID = 64
    NCLS = 40
    NT = 99               # dst tiles per core (128 lanes each)
    KSEG = 4              # chunks (of 128 edges) per (quarter, tile) segment
    SLABC = 11            # chunks per gather slab
    SINGLE_PACKET = False  # multi-packet gathers (single-packet hangs >~1K idxs)
    NQUEUES = 4           # spread gathers over all 4 SWDGE queues
    MSGBUFS = 25
    IDXBUFS = 1
    GE = 5                # tiles per epilogue-matmul batch
    POOLV = 1             # of 16 chunks/tile, how many V-builds go to Pool
    PSB = 3               # psum accumulator ring depth
    VPB = 8               # V-tile ring depth
    LNG = 5               # tiles per deferred-Ln group
    NZBIAS = False        # set per-input: any of b1/b2/bc nonzero

    @property
    def PADSHARD(self):
        return self.NT * 128

    @property
    def NPAD(self):
        return self.PADSHARD * self.M

    @property
    def QROWS(self):
        return self.NPAD // 4

    @property
    def SEG(self):
        return self.KSEG * 128

    @property
    def CQ(self):
        return self.NT * self.KSEG          # chunks per quarter

    @property
    def NSLAB(self):
        assert self.CQ % self.SLABC == 0
        return self.CQ // self.SLABC        # gather slabs per quarter

    @property
    def CHUNKS(self):
        return 4 * self.CQ


CFG = Cfg()


# ------------------------------------------------------------- host plan ---
def _assign_slots(cfg, edge_row, edge_col):
    """Assign nodes to table slots so every (src-quarter, dst-tile) edge
    bucket holds <= KSEG*128 edges. Returns slot_of[node] -> [0, NPAD).

    Nodes are first split into 4 fixed quarter groups (so each node's
    src-quarter is pinned), then greedily packed into the 2*NT tiles of
    their own quarter balancing the 4-vector of per-src-quarter in-edge
    counts.
    """
    N, NPAD, QROWS, NT, M = cfg.N_NODES, cfg.NPAD, cfg.QROWS, cfg.NT, cfg.M
    TPQ = QROWS // 128                       # tiles per quarter (2 cores)
    rng = np.random.default_rng(12345)
    order = rng.permutation(N)
    qgrp = np.empty(N, dtype=np.int64)       # node -> quarter group
    npq = N // 4
    for q in range(4):
        qgrp[order[q * npq:(q + 1) * npq]] = q
    qgrp[order[4 * npq:]] = 3

    # per-node in-edge count by source quarter
    cnt = np.zeros((N, 4), dtype=np.int64)
    np.add.at(cnt, (edge_row, qgrp[edge_col]), 1)

    slot_of = np.empty(N, dtype=np.int64)
    for q in range(4):
        nodes = np.where(qgrp == q)[0]
        c = cnt[nodes].astype(np.float32)            # [nq, 4]
        tot = c.sum(1)
        o = np.argsort(-tot, kind="stable")
        nodes, c = nodes[o], c[o]
        loads = np.zeros((TPQ, 4), dtype=np.float32)
        fill = np.zeros(TPQ, dtype=np.int64)
        pos = np.empty(nodes.size, dtype=np.int64)
        for i in range(nodes.size):
            cand = np.max(loads + c[i], axis=1) + (fill >= 128) * 1e9
            b = int(np.argmin(cand))
            loads[b] += c[i]
            pos[i] = b * 128 + fill[b]
            fill[b] += 1
        slot_of[nodes] = q * QROWS + pos
    return slot_of


def _plan(cfg, edge_row, edge_col, edge_val, slot_of):
    """Bucket/sort/pad edges per core. Returns per-core arrays:
    idx16 [128, 4*CQ*128/16] int16, ldstT/valT [128, CHUNKS] bf16,
    plus degs [128, NT] f32 per core.
    """
    M, NT, KSEG, SEG, CQ, QROWS = cfg.M, cfg.NT, cfg.KSEG, cfg.SEG, cfg.CQ, cfg.QROWS
    PADSHARD = cfg.PADSHARD

    src_slot = slot_of[edge_col]
    dst_slot = slot_of[edge_row]
    q_of = src_slot // QROWS
    i_of = src_slot % QROWS
    core_of = dst_slot // PADSHARD
    dloc = dst_slot % PADSHARD
    t_of = dloc // 128
    l_of = dloc % 128

    deg = np.zeros(cfg.NPAD, dtype=np.float64)
    np.add.at(deg, dst_slot, edge_val.astype(np.float64))

    L = 4 * CQ * 128
    idx_all, ldst_all, val_all, deg_all = [], [], [], []
    for c in range(M):
        sel = core_of == c
        segid = q_of[sel] * NT + t_of[sel]
        order = np.argsort(segid, kind="stable")
        sid = segid[order]
        idx_s = i_of[sel][order]
        l_s = l_of[sel][order]
        v_s = edge_val[sel][order]

        counts = np.bincount(sid, minlength=4 * NT)
        if counts.max() > SEG:
            raise ValueError(f"segment overflow: {counts.max()} > {SEG}")
        starts = np.arange(4 * NT) * SEG
        pos = starts[sid] + (np.arange(sid.size)
                             - np.concatenate(([0], np.cumsum(counts)))[sid])

        idx = np.zeros(L, dtype=np.int16)
        ldst = np.zeros(L, dtype=np.float32)
        val = np.zeros(L, dtype=np.float32)
        idx[pos] = idx_s.astype(np.int16)
        ldst[pos] = l_s.astype(np.float32)
        val[pos] = v_s.astype(np.float32)

        # wrap indices: idx i -> [i%16, i//16], replicated on all 8 q7 cores
        idxw = np.tile(idx.reshape(-1, 16).T, (8, 1)).copy()
        ldstT = np.ascontiguousarray(ldst.reshape(-1, 128).T)
        valT = np.ascontiguousarray(val.reshape(-1, 128).T)
        degs = np.ascontiguousarray(
            deg[c * PADSHARD:(c + 1) * PADSHARD].reshape(NT, 128).T
        ).astype(np.float32)
        idx_all.append(idxw)
        ldst_all.append(ldstT)
        val_all.append(valT)
        deg_all.append(degs)
    return idx_all, ldst_all, val_all, deg_all


# --------------------------------------------------------- device program ---
def _build(cfg, timing=False, nzbias=False):
    import os
    from concourse import bacc, tile
    import concourse.mybir as mybir
    kdbg = bool(os.environ.get("KDBG"))

    f32 = mybir.dt.float32
    bf16 = mybir.dt.bfloat16
    i16 = mybir.dt.int16
    AOP = mybir.AluOpType
    ACT = mybir.ActivationFunctionType

    nc = bacc.Bacc("TRN2", target_bir_lowering=False, debug=False,
                   num_devices=1 if timing else cfg.M,
                   dynamic_dma_scratch_size=16384,
                   num_swdge_queues=cfg.NQUEUES)

    NPAD, QROWS, NT, KSEG, CQ, SLABC, NSLAB = (
        cfg.NPAD, cfg.QROWS, cfg.NT, cfg.KSEG, cfg.CQ, cfg.SLABC, cfg.NSLAB)
    CHUNKS, HID, NCLS, IN_DIM = cfg.CHUNKS, cfg.HID, cfg.NCLS, cfg.IN_DIM
    LQ16 = CQ * 128 // 16              # idx columns per quarter
    SLAB16 = SLABC * 128 // 16         # idx columns per slab

    # -------- I/O
    XG = nc.dram_tensor("xg", [NPAD, IN_DIM], bf16, kind="ExternalInput")
    IDX = nc.dram_tensor("idx", [128, 4 * LQ16], i16, kind="ExternalInput")
    LDST = nc.dram_tensor("ldst", [128, CHUNKS], f32, kind="ExternalInput")
    VAL = nc.dram_tensor("val", [128, CHUNKS], f32, kind="ExternalInput")
    DEG = nc.dram_tensor("deg", [128, NT], f32, kind="ExternalInput")
    W1 = nc.dram_tensor("w1", [IN_DIM, HID], bf16, kind="ExternalInput")
    W2C = nc.dram_tensor("w2c", [HID, NCLS], bf16, kind="ExternalInput")
    B1R = nc.dram_tensor("b1r", [128, HID], f32, kind="ExternalInput")
    BCOMBR = nc.dram_tensor("bcombr", [128, NCLS], f32, kind="ExternalInput")
    BCR = nc.dram_tensor("bcr", [128, NCLS], f32, kind="ExternalInput")
    IOTA = nc.dram_tensor("iota", [128, 128], bf16, kind="ExternalInput")
    f16 = mybir.dt.float16
    OUT = nc.dram_tensor("out", [cfg.PADSHARD, NCLS], f16, kind="ExternalOutput")
    HDBG = (nc.dram_tensor("hdbg", [cfg.PADSHARD, HID], bf16,
                           kind="ExternalOutput") if kdbg else None)
    LDBG = (nc.dram_tensor("ldbg", [128, NT * NCLS], f32,
                           kind="ExternalOutput") if kdbg else None)
    SMDBG = (nc.dram_tensor("smdbg", [128, NT], f32,
                            kind="ExternalOutput") if kdbg else None)

    # -------- internal DRAM
    HS = nc.dram_tensor("hshard", [cfg.PADSHARD, 128], bf16)    # cols 64+: junk
    HF = nc.dram_tensor("hfull", [NPAD, 128], bf16, addr_space="Shared")

    with tile.TileContext(nc) as tc, ExitStack() as top:
        # idx quarter 0 + V-build operands first: the first gathers and
        # V-builds depend only on these, so they issue before the consts.
        # one idx tile set serves BOTH layers (identical edge plan)
        idxp = top.enter_context(tc.tile_pool(name="idxp", bufs=1))
        its = [None] * 4
        its[0] = idxp.tile([128, LQ16], i16, tag="idx_0", name="idx_0")
        nc.sync.dma_start(out=its[0], in_=IDX[:, 0:LQ16])

        cpool = top.enter_context(tc.tile_pool(name="consts", bufs=1))
        iot = cpool.tile([128, 128], bf16)
        nc.sync.dma_start(out=iot, in_=IOTA[:, :])

        edg = top.enter_context(tc.tile_pool(name="edg", bufs=1))
        ldsts = edg.tile([128, CHUNKS], f32)
        nc.sync.dma_start(out=ldsts, in_=LDST[:, :])
        vals = edg.tile([128, CHUNKS], f32)
        nc.sync.dma_start(out=vals, in_=VAL[:, :])

        for q in range(1, 4):
            its[q] = idxp.tile([128, LQ16], i16, tag=f"idx_{q}",
                               name=f"idx_{q}")
            nc.sync.dma_start(out=its[q],
                              in_=IDX[:, q * LQ16:(q + 1) * LQ16])

        w1s = cpool.tile([IN_DIM, HID], bf16)
        nc.sync.dma_start(out=w1s, in_=W1[:, :])
        w2cs = cpool.tile([HID, NCLS], bf16)
        nc.sync.dma_start(out=w2cs, in_=W2C[:, :])
        b1rs = cpool.tile([128, HID], f32)
        nc.sync.dma_start(out=b1rs, in_=B1R[:, :])
        bcombs = cpool.tile([128, NCLS], f32)
        nc.sync.dma_start(out=bcombs, in_=BCOMBR[:, :])
        bcrs = cpool.tile([128, NCLS], f32)
        nc.sync.dma_start(out=bcrs, in_=BCR[:, :])
        degs = cpool.tile([128, NT], f32)
        nc.sync.dma_start(out=degs, in_=DEG[:, :])


        # ============ spmm layer runner: per-tile single psum group across
        # all 4 quarters, accumulating transposed (psT = Xg^T V); epilogue
        # split into a per-tile part (cast) and a batched per-GE-tiles part
        # (weight matmuls etc) to keep the PE stream free of cross-engine
        # round trips.
        msg = top.enter_context(tc.tile_pool(name="msg", bufs=cfg.MSGBUFS))
        vp = top.enter_context(tc.tile_pool(name="vp", bufs=cfg.VPB))
        psb = top.enter_context(
            tc.tile_pool(name="psb", bufs=cfg.PSB, space="PSUM"))

        def spmm_layer(tab, width, lits, epi_tile, epi_group, ltag, gplan):
            gend = {}
            acc = 0
            for gsz in gplan:
                acc += gsz
                gend[acc - 1] = gsz
            assert acc == NT
            slabs = [[None] * NSLAB for _ in range(4)]

            def ensure_slab(q, s):
                if slabs[q][s] is None:
                    mt = msg.tile([128, SLABC, 128], bf16)
                    nc.gpsimd.dma_gather(
                        mt, tab[q * QROWS:(q + 1) * QROWS, :],
                        lits[q][:, s * SLAB16:(s + 1) * SLAB16],
                        num_idxs=SLABC * 128, num_idxs_reg=SLABC * 128,
                        elem_size=128, elem_step=128,
                        single_packet=cfg.SINGLE_PACKET,
                        queue_num=(q * NSLAB + s) % cfg.NQUEUES)
                    slabs[q][s] = mt
                return slabs[q][s]

            for t in range(NT):
                psfull = psb.tile([128, 128], f32, tag="ps", name="ps")
                ps = psfull if width == 128 else psfull[0:width, :]
                for q in range(4):
                    j0 = t * KSEG
                    vt = vp.tile([128, KSEG, 128], bf16)
                    for k in range(KSEG):
                        gj = q * CQ + j0 + k             # global chunk
                        veng = (nc.gpsimd
                                if q * KSEG + k >= 16 - cfg.POOLV
                                else nc.vector)
                        veng.tensor_scalar(
                            vt[:, k, :], iot, ldsts[:, gj:gj + 1],
                            vals[:, gj:gj + 1], AOP.is_equal, AOP.mult)
                    for k in range(KSEG):
                        j = j0 + k                       # chunk in quarter
                        mt = ensure_slab(q, j // SLABC)
                        nc.tensor.matmul(ps, lhsT=mt[:, j % SLABC, 0:width],
                                         rhs=vt[:, k, :],
                                         start=(q == 0 and k == 0),
                                         stop=(q == 3 and k == KSEG - 1))
                epi_tile(t, ps)
                if t in gend:
                    epi_group(t - gend[t] + 1, gend[t])

        # ================= layer 1: h = relu((A x)@W1 + deg*b1^T), store bf16
        with tc.tile_pool(name="tc1", bufs=cfg.GE + 2) as tp1, \
             tc.tile_pool(name="tc2", bufs=3) as tp2, \
             tc.tile_pool(name="pse", bufs=3, space="PSUM") as pse:
            pss1 = {}

            def epi1_tile(t, ps):
                pss = tp1.tile([IN_DIM, 128], bf16, tag="pss", name="pss")
                nc.scalar.activation(pss, ps, ACT.Copy)
                pss1[t] = pss

            def epi1_group(t0, n):
                for t in range(t0, t0 + n):
                    ph = pse.tile([128, HID], f32)
                    nc.tensor.matmul(ph, lhsT=pss1.pop(t), rhs=w1s,
                                     start=True, stop=True)
                    ht = tp2.tile([128, HID], bf16, tag="ht")
                    if nzbias:
                        tb = tp2.tile([128, HID], f32, tag="tb")
                        nc.vector.tensor_scalar(tb, b1rs, degs[:, t:t + 1],
                                                None, AOP.mult)
                        hsum = tp2.tile([128, HID], f32, tag="hsum")
                        nc.vector.tensor_tensor(hsum, ph, tb, AOP.add)
                        nc.scalar.activation(ht, hsum, ACT.Relu)
                    else:
                        nc.scalar.activation(ht, ph, ACT.Relu)
                    nc.sync.dma_start(out=HS[t * 128:(t + 1) * 128, 0:HID],
                                      in_=ht)

            spmm_layer(XG, IN_DIM, its, epi1_tile, epi1_group, "a",
                       [cfg.GE] * (NT // cfg.GE)
                       + ([NT % cfg.GE] if NT % cfg.GE else []))
            if kdbg:
                nc.sync.dma_start(out=HDBG[:, :], in_=HS[:, 0:HID])
            if not timing:
                nc.gpsimd.collective_compute(
                    "AllGather", mybir.AluOpType.bypass,
                    replica_groups=[list(range(cfg.M))],
                    ins=[HS[:, :]], outs=[HF[:, :]])

        # ================= layer 2 + fused classifier/log_softmax
        with tc.tile_pool(name="te1", bufs=cfg.GE + 2) as te1, \
             tc.tile_pool(name="te2", bufs=3) as te2, \
             tc.tile_pool(name="te3", bufs=2) as te3, \
             tc.tile_pool(name="psf", bufs=3, space="PSUM") as psf:
            G = cfg.LNG
            _rem = NT - cfg.LNG * (NT // cfg.LNG - 1)
            gplan2 = [cfg.LNG] * (NT // cfg.LNG - 1) + [_rem - 5, 3, 2]
            gname = {}
            acc = 0
            for gi, gsz in enumerate(gplan2):
                for i in range(gsz):
                    gname[acc + i] = (gi, i, gsz)
                acc += gsz
            pss2 = {}
            state = {}

            def epi2_tile(t, ps):
                pss = te1.tile([HID, 128], bf16, tag="pss", name="pss")
                nc.scalar.activation(pss, ps, ACT.Copy)
                pss2[t] = pss

            def epi2_group(t0, n):
                for t in range(t0, t0 + n):
                    g, i, gsz = gname[t]
                    if i == 0:
                        state["lgg"] = te3.tile([128, G, NCLS], f32,
                                                tag="lgg", name="lgg")
                        state["negg"] = te3.tile([128, G], f32,
                                                 tag="negg", name="negg")
                        state["smg"] = te3.tile([128, G], f32,
                                                tag="smg", name="smg")
                    lgg, negg, smg = state["lgg"], state["negg"], state["smg"]
                    psl = psf.tile([128, NCLS], f32)
                    nc.tensor.matmul(psl, lhsT=pss2.pop(t), rhs=w2cs,
                                     start=True, stop=True)
                    if nzbias:
                        tb = te2.tile([128, NCLS], f32, tag="tb")
                        nc.vector.tensor_scalar(tb, bcombs, degs[:, t:t + 1],
                                                None, AOP.mult)
                        lg0 = te2.tile([128, NCLS], f32, tag="lg0")
                        nc.vector.tensor_tensor(lg0, psl, tb, AOP.add)
                        nc.gpsimd.tensor_tensor(lgg[:, i, :], lg0, bcrs,
                                                AOP.add)
                    else:
                        nc.scalar.activation(lgg[:, i, :], psl, ACT.Copy)
                    if i == gsz - 1:
                        if kdbg:
                            nc.sync.dma_start(
                                out=LDBG[:, g * G * NCLS:(g + 1) * G * NCLS],
                                in_=lgg.rearrange("p a b -> p (a b)"))
                            nc.sync.dma_start(out=SMDBG[:, g * G:(g + 1) * G],
                                              in_=smg)
                        nc.vector.tensor_reduce(negg[:, 0:gsz],
                                                lgg[:, 0:gsz, :],
                                                mybir.AxisListType.X, AOP.max,
                                                negate=True)
                        for ii in range(gsz):
                            et = te2.tile([128, NCLS], f32, tag="et")
                            nc.scalar.activation(et, lgg[:, ii, :], ACT.Exp,
                                                 bias=negg[:, ii:ii + 1],
                                                 accum_out=smg[:, ii:ii + 1])
                        lng = te2.tile([128, G], f32, tag="lng")
                        nc.scalar.activation(lng[:, 0:gsz], smg[:, 0:gsz],
                                             ACT.Ln)
                        shg = te2.tile([128, G], f32, tag="shg")
                        nc.vector.tensor_tensor(shg[:, 0:gsz], negg[:, 0:gsz],
                                                lng[:, 0:gsz], AOP.subtract)
                        for ii in range(gsz):
                            tt = t - gsz + 1 + ii
                            ot = te2.tile([128, NCLS], f16, tag="ot")
                            nc.vector.tensor_scalar(ot, lgg[:, ii, :],
                                                    shg[:, ii:ii + 1], None,
                                                    AOP.add)
                            nc.sync.dma_start(
                                out=OUT[tt * 128:(tt + 1) * 128, :], in_=ot)

            spmm_layer(HF, HID, its, epi2_tile, epi2_group, "b", gplan2)

    nc.compile()
    return nc


_NC_CACHE = {}


def _get_nc(cfg):
    key = (cfg.NT, cfg.KSEG, cfg.SLABC, cfg.NZBIAS)
    if key not in _NC_CACHE:
        _NC_CACHE[key] = _build(cfg, nzbias=cfg.NZBIAS)
    return _NC_CACHE[key]


# ------------------------------------------------------------------ main ---
def kernel(x, edge_row, edge_col, edge_val, W1, b1, W2, b2, Wc, bc,
           _run_kwargs=None):
    from concourse.bass_utils import run_bass_kernel_spmd

    cfg = CFG
    x = np.asarray(x, dtype=np.float32)
    edge_row = np.asarray(edge_row, dtype=np.int64)
    edge_col = np.asarray(edge_col, dtype=np.int64)
    edge_val = np.asarray(edge_val, dtype=np.float32)
    W1 = np.asarray(W1, dtype=np.float32)
    W2 = np.asarray(W2, dtype=np.float32)
    Wc = np.asarray(Wc, dtype=np.float32)
    b1 = np.asarray(b1, dtype=np.float32)
    b2 = np.asarray(b2, dtype=np.float32)
    bc = np.asarray(bc, dtype=np.float32)

    cfg.NZBIAS = bool(np.any(b1) or np.any(b2) or np.any(bc))
    slot_of = _assign_slots(cfg, edge_row, edge_col)
    try:
        idx_all, ldst_all, val_all, deg_all = _plan(
            cfg, edge_row, edge_col, edge_val, slot_of)
    except ValueError:
        cfg.KSEG += 1
        idx_all, ldst_all, val_all, deg_all = _plan(
            cfg, edge_row, edge_col, edge_val, slot_of)

    xg = np.zeros((cfg.NPAD, cfg.IN_DIM), dtype=ml_dtypes.bfloat16)
    xg[slot_of] = x.astype(ml_dtypes.bfloat16)

    w1h = W1.astype(ml_dtypes.bfloat16)
    w2c = (W2 @ Wc).astype(ml_dtypes.bfloat16)
    bcomb = b2 @ Wc
    iota = np.tile(np.arange(128, dtype=np.float32), (128, 1)).astype(
        ml_dtypes.bfloat16)
    b1r = np.tile(b1, (128, 1)).astype(np.float32)
    bcombr = np.tile(bcomb, (128, 1)).astype(np.float32)
    bcr = np.tile(bc, (128, 1)).astype(np.float32)

    nc = _get_nc(cfg)
    in_maps = []
    for c in range(cfg.M):
        in_maps.append({
            "xg": xg, "idx": idx_all[c], "ldst": ldst_all[c],
            "val": val_all[c], "deg": deg_all[c], "w1": w1h, "w2c": w2c,
            "b1r": b1r, "bcombr": bcombr, "bcr": bcr, "iota": iota,
        })
    kw = dict(_run_kwargs or {})
    res = run_bass_kernel_spmd(nc, in_maps, core_ids=list(range(cfg.M)), **kw)
    shard = np.concatenate(
        [res.results[c]["out"] for c in range(cfg.M)], axis=0)  # [NPAD, NCLS]
    out = shard[slot_of]
    kernel.last_results = res
    return out.astype(np.float32)


# revision 43
# speedup vs baseline: 1.0176x; 1.0079x over previous
"""GCN node classifier (2x spmm + classifier + log_softmax) on 8 trn2 cores.

Strategy: destination-node 1D sharding with spmm linearity.
  spmm(A, x@W1 + b1) = (A x)@W1 + deg * b1^T      (deg = rowsum of A)
  spmm(A, h@W2 + b2)@Wc = (A h)@(W2 Wc) + deg * (b2 Wc)^T
so the gather tables are the RAW node features (x bf16 for layer 1,
relu-h bf16 for layer 2) — no dense pre-pass over all nodes, and the
layer weights are applied per dst tile after aggregation.

Each core owns 12,800 dst slots (100 tiles x 128 lanes). Host assigns
nodes to slots with a greedy 4-d balancer so that every (src-quarter,
dst-tile) edge bucket fits in KSEG=4 chunks of 128 edges (the int16
gather index forces 4 quarter views of the 102,400-row table). Per-edge
source rows are fetched with GPSIMD dma_gather (256B rows); the
segment-sum is a tensor-engine matmul against per-chunk scatter
matrices V[e, dst_lane] = edge_val[e] built on DVE with
(iota == ldst) * val, accumulated transposed (psT = Xg^T V) so the
per-tile epilogue can feed psT straight back as lhsT for the weight
matmul. log-softmax is fused per tile. Between layers the per-shard
relu-h table is AllGather'ed into a Shared DRAM tensor.
"""

import numpy as np
import ml_dtypes

from contextlib import ExitStack


# ---------------------------------------------------------------- config ---
class Cfg:
    M = 8                 # cores
    N_NODES = 100000
    N_EDGES = 1600000
    IN_DIM = 128
    HID = 64
    NCLS = 40
    NT = 99               # dst tiles per core (128 lanes each)
    KSEG = 4              # chunks (of 128 edges) per (quarter, tile) segment
    SLABC = 9             # chunks per gather slab
    SINGLE_PACKET = False  # multi-packet gathers (single-packet hangs >~1K idxs)
    NQUEUES = 4           # spread gathers over all 4 SWDGE queues
    MSGBUFS = 30
    IDXBUFS = 1
    GE = 5                # tiles per epilogue-matmul batch
    POOLV = 1             # of 16 chunks/tile, how many V-builds go to Pool
    PSB = 3               # psum accumulator ring depth
    VPB = 8               # V-tile ring depth
    LNG = 5               # tiles per deferred-Ln group
    NZBIAS = False        # set per-input: any of b1/b2/bc nonzero

    @property
    def PADSHARD(self):
        return self.NT * 128

    @property
    def NPAD(self):
        return self.PADSHARD * self.M

    @property
    def QROWS(self):
        return self.NPAD // 4

    @property
    def SEG(self):
        return self.KSEG * 128

    @property
    def CQ(self):
        return self.NT * self.KSEG          # chunks per quarter

    @property
    def NSLAB(self):
        assert self.CQ % self.SLABC == 0
        return self.CQ // self.SLABC        # gather slabs per quarter

    @property
    def CHUNKS(self):
        return 4 * self.CQ


CFG = Cfg()


# ------------------------------------------------------------- host plan ---
def _assign_slots(cfg, edge_row, edge_col):
    """Assign nodes to table slots so every (src-quarter, dst-tile) edge
    bucket holds <= KSEG*128 edges. Returns slot_of[node] -> [0, NPAD).

    Nodes are first split into 4 fixed quarter groups (so each node's
    src-quarter is pinned), then greedily packed into the 2*NT tiles of
    their own quarter balancing the 4-vector of per-src-quarter in-edge
    counts.
    """
    N, NPAD, QROWS, NT, M = cfg.N_NODES, cfg.NPAD, cfg.QROWS, cfg.NT, cfg.M
    TPQ = QROWS // 128                       # tiles per quarter (2 cores)
    rng = np.random.default_rng(12345)
    order = rng.permutation(N)
    qgrp = np.empty(N, dtype=np.int64)       # node -> quarter group
    npq = N // 4
    for q in range(4):
        qgrp[order[q * npq:(q + 1) * npq]] = q
    qgrp[order[4 * npq:]] = 3

    # per-node in-edge count by source quarter
    cnt = np.zeros((N, 4), dtype=np.int64)
    np.add.at(cnt, (edge_row, qgrp[edge_col]), 1)

    slot_of = np.empty(N, dtype=np.int64)
    for q in range(4):
        nodes = np.where(qgrp == q)[0]
        c = cnt[nodes].astype(np.float32)            # [nq, 4]
        tot = c.sum(1)
        o = np.argsort(-tot, kind="stable")
        nodes, c = nodes[o], c[o]
        loads = np.zeros((TPQ, 4), dtype=np.float32)
        fill = np.zeros(TPQ, dtype=np.int64)
        pos = np.empty(nodes.size, dtype=np.int64)
        for i in range(nodes.size):
            cand = np.max(loads + c[i], axis=1) + (fill >= 128) * 1e9
            b = int(np.argmin(cand))
            loads[b] += c[i]
            pos[i] = b * 128 + fill[b]
            fill[b] += 1
        slot_of[nodes] = q * QROWS + pos
    return slot_of


def _plan(cfg, edge_row, edge_col, edge_val, slot_of):
    """Bucket/sort/pad edges per core. Returns per-core arrays:
    idx16 [128, 4*CQ*128/16] int16, ldstT/valT [128, CHUNKS] bf16,
    plus degs [128, NT] f32 per core.
    """
    M, NT, KSEG, SEG, CQ, QROWS = cfg.M, cfg.NT, cfg.KSEG, cfg.SEG, cfg.CQ, cfg.QROWS
    PADSHARD = cfg.PADSHARD

    src_slot = slot_of[edge_col]
    dst_slot = slot_of[edge_row]
    q_of = src_slot // QROWS
    i_of = src_slot % QROWS
    core_of = dst_slot // PADSHARD
    dloc = dst_slot % PADSHARD
    t_of = dloc // 128
    l_of = dloc % 128

    deg = np.zeros(cfg.NPAD, dtype=np.float64)
    np.add.at(deg, dst_slot, edge_val.astype(np.float64))

    L = 4 * CQ * 128
    idx_all, ldst_all, val_all, deg_all = [], [], [], []
    for c in range(M):
        sel = core_of == c
        segid = q_of[sel] * NT + t_of[sel]
        order = np.argsort(segid, kind="stable")
        sid = segid[order]
        idx_s = i_of[sel][order]
        l_s = l_of[sel][order]
        v_s = edge_val[sel][order]

        counts = np.bincount(sid, minlength=4 * NT)
        if counts.max() > SEG:
            raise ValueError(f"segment overflow: {counts.max()} > {SEG}")
        starts = np.arange(4 * NT) * SEG
        pos = starts[sid] + (np.arange(sid.size)
                             - np.concatenate(([0], np.cumsum(counts)))[sid])

        idx = np.zeros(L, dtype=np.int16)
        ldst = np.zeros(L, dtype=np.float32)
        val = np.zeros(L, dtype=np.float32)
        idx[pos] = idx_s.astype(np.int16)
        ldst[pos] = l_s.astype(np.float32)
        val[pos] = v_s.astype(np.float32)

        # wrap indices: idx i -> [i%16, i//16], replicated on all 8 q7 cores
        idxw = np.tile(idx.reshape(-1, 16).T, (8, 1)).copy()
        ldstT = np.ascontiguousarray(ldst.reshape(-1, 128).T)
        valT = np.ascontiguousarray(val.reshape(-1, 128).T)
        degs = np.ascontiguousarray(
            deg[c * PADSHARD:(c + 1) * PADSHARD].reshape(NT, 128).T
        ).astype(np.float32)
        idx_all.append(idxw)
        ldst_all.append(ldstT)
        val_all.append(valT)
        deg_all.append(degs)
    return idx_all, ldst_all, val_all, deg_all


# --------------------------------------------------------- device program ---
def _build(cfg, timing=False, nzbias=False):
    import os
    from concourse import bacc, tile
    import concourse.mybir as mybir
    kdbg = bool(os.environ.get("KDBG"))

    f32 = mybir.dt.float32
    bf16 = mybir.dt.bfloat16
    i16 = mybir.dt.int16
    AOP = mybir.AluOpType
    ACT = mybir.ActivationFunctionType

    nc = bacc.Bacc("TRN2", target_bir_lowering=False, debug=False,
                   num_devices=1 if timing else cfg.M,
                   dynamic_dma_scratch_size=16384,
                   num_swdge_queues=cfg.NQUEUES)

    NPAD, QROWS, NT, KSEG, CQ, SLABC, NSLAB = (
        cfg.NPAD, cfg.QROWS, cfg.NT, cfg.KSEG, cfg.CQ, cfg.SLABC, cfg.NSLAB)
    CHUNKS, HID, NCLS, IN_DIM = cfg.CHUNKS, cfg.HID, cfg.NCLS, cfg.IN_DIM
    LQ16 = CQ * 128 // 16              # idx columns per quarter
    SLAB16 = SLABC * 128 // 16         # idx columns per slab

    # -------- I/O
    XG = nc.dram_tensor("xg", [NPAD, IN_DIM], bf16, kind="ExternalInput")
    IDX = nc.dram_tensor("idx", [128, 4 * LQ16], i16, kind="ExternalInput")
    LDST = nc.dram_tensor("ldst", [128, CHUNKS], f32, kind="ExternalInput")
    VAL = nc.dram_tensor("val", [128, CHUNKS], f32, kind="ExternalInput")
    DEG = nc.dram_tensor("deg", [128, NT], f32, kind="ExternalInput")
    W1 = nc.dram_tensor("w1", [IN_DIM, HID], bf16, kind="ExternalInput")
    W2C = nc.dram_tensor("w2c", [HID, NCLS], bf16, kind="ExternalInput")
    B1R = nc.dram_tensor("b1r", [128, HID], f32, kind="ExternalInput")
    BCOMBR = nc.dram_tensor("bcombr", [128, NCLS], f32, kind="ExternalInput")
    BCR = nc.dram_tensor("bcr", [128, NCLS], f32, kind="ExternalInput")
    IOTA = nc.dram_tensor("iota", [128, 128], bf16, kind="ExternalInput")
    f16 = mybir.dt.float16
    OUT = nc.dram_tensor("out", [cfg.PADSHARD, NCLS], f16, kind="ExternalOutput")
    HDBG = (nc.dram_tensor("hdbg", [cfg.PADSHARD, HID], bf16,
                           kind="ExternalOutput") if kdbg else None)
    LDBG = (nc.dram_tensor("ldbg", [128, NT * NCLS], f32,
                           kind="ExternalOutput") if kdbg else None)
    SMDBG = (nc.dram_tensor("smdbg", [128, NT], f32,
                            kind="ExternalOutput") if kdbg else None)

    # -------- internal DRAM
    HS = nc.dram_tensor("hshard", [cfg.PADSHARD, 128], bf16)    # cols 64+: junk
    HF = nc.dram_tensor("hfull", [NPAD, 128], bf16, addr_space="Shared")

    with tile.TileContext(nc) as tc, ExitStack() as top:
        # idx quarter 0 + V-build operands first: the first gathers and
        # V-builds depend only on these, so they issue before the consts.
        # one idx tile set serves BOTH layers (identical edge plan)
        idxp = top.enter_context(tc.tile_pool(name="idxp", bufs=1))
        its = [None] * 4
        its[0] = idxp.tile([128, LQ16], i16, tag="idx_0", name="idx_0")
        nc.sync.dma_start(out=its[0], in_=IDX[:, 0:LQ16])

        cpool = top.enter_context(tc.tile_pool(name="consts", bufs=1))
        iot = cpool.tile([128, 128], bf16)
        nc.sync.dma_start(out=iot, in_=IOTA[:, :])

        edg = top.enter_context(tc.tile_pool(name="edg", bufs=1))
        ldsts = edg.tile([128, CHUNKS], f32)
        nc.sync.dma_start(out=ldsts, in_=LDST[:, :])
        vals = edg.tile([128, CHUNKS], f32)
        nc.sync.dma_start(out=vals, in_=VAL[:, :])

        for q in range(1, 4):
            its[q] = idxp.tile([128, LQ16], i16, tag=f"idx_{q}",
                               name=f"idx_{q}")
            nc.sync.dma_start(out=its[q],
                              in_=IDX[:, q * LQ16:(q + 1) * LQ16])

        w1s = cpool.tile([IN_DIM, HID], bf16)
        nc.sync.dma_start(out=w1s, in_=W1[:, :])
        w2cs = cpool.tile([HID, NCLS], bf16)
        nc.sync.dma_start(out=w2cs, in_=W2C[:, :])
        b1rs = bcombs = bcrs = degs = None
        if nzbias:
            b1rs = cpool.tile([128, HID], f32)
            nc.sync.dma_start(out=b1rs, in_=B1R[:, :])
            bcombs = cpool.tile([128, NCLS], f32)
            nc.sync.dma_start(out=bcombs, in_=BCOMBR[:, :])
            bcrs = cpool.tile([128, NCLS], f32)
            nc.sync.dma_start(out=bcrs, in_=BCR[:, :])
            degs = cpool.tile([128, NT], f32)
            nc.sync.dma_start(out=degs, in_=DEG[:, :])


        # ============ spmm layer runner: per-tile single psum group across
        # all 4 quarters, accumulating transposed (psT = Xg^T V); epilogue
        # split into a per-tile part (cast) and a batched per-GE-tiles part
        # (weight matmuls etc) to keep the PE stream free of cross-engine
        # round trips.
        msg = top.enter_context(tc.tile_pool(name="msg", bufs=cfg.MSGBUFS))
        vp = top.enter_context(tc.tile_pool(name="vp", bufs=cfg.VPB))
        psb = top.enter_context(
            tc.tile_pool(name="psb", bufs=cfg.PSB, space="PSUM"))

        def spmm_layer(tab, width, lits, epi_tile, epi_group, ltag, gplan):
            gend = {}
            acc = 0
            for gsz in gplan:
                acc += gsz
                gend[acc - 1] = gsz
            assert acc == NT
            slabs = [[None] * NSLAB for _ in range(4)]

            def ensure_slab(q, s):
                if slabs[q][s] is None:
                    mt = msg.tile([128, SLABC, 128], bf16)
                    nc.gpsimd.dma_gather(
                        mt, tab[q * QROWS:(q + 1) * QROWS, :],
                        lits[q][:, s * SLAB16:(s + 1) * SLAB16],
                        num_idxs=SLABC * 128, num_idxs_reg=SLABC * 128,
                        elem_size=128, elem_step=128,
                        single_packet=cfg.SINGLE_PACKET,
                        queue_num=(q * NSLAB + s) % cfg.NQUEUES)
                    slabs[q][s] = mt
                return slabs[q][s]

            for t in range(NT):
                psfull = psb.tile([128, 128], f32, tag="ps", name="ps")
                ps = psfull if width == 128 else psfull[0:width, :]
                for q in range(4):
                    j0 = t * KSEG
                    vt = vp.tile([128, KSEG, 128], bf16)
                    for k in range(KSEG):
                        gj = q * CQ + j0 + k             # global chunk
                        veng = (nc.gpsimd
                                if q * KSEG + k >= 16 - cfg.POOLV
                                else nc.vector)
                        veng.tensor_scalar(
                            vt[:, k, :], iot, ldsts[:, gj:gj + 1],
                            vals[:, gj:gj + 1], AOP.is_equal, AOP.mult)
                    for k in range(KSEG):
                        j = j0 + k                       # chunk in quarter
                        mt = ensure_slab(q, j // SLABC)
                        nc.tensor.matmul(ps, lhsT=mt[:, j % SLABC, 0:width],
                                         rhs=vt[:, k, :],
                                         start=(q == 0 and k == 0),
                                         stop=(q == 3 and k == KSEG - 1))
                epi_tile(t, ps)
                if t in gend:
                    epi_group(t - gend[t] + 1, gend[t])

        # ================= layer 1: h = relu((A x)@W1 + deg*b1^T), store bf16
        with tc.tile_pool(name="tc1", bufs=cfg.GE + 2) as tp1, \
             tc.tile_pool(name="tc2", bufs=3) as tp2, \
             tc.tile_pool(name="pse", bufs=3, space="PSUM") as pse:
            pss1 = {}

            def epi1_tile(t, ps):
                pss = tp1.tile([IN_DIM, 128], bf16, tag="pss", name="pss")
                nc.scalar.activation(pss, ps, ACT.Copy)
                pss1[t] = pss

            def epi1_group(t0, n):
                for t in range(t0, t0 + n):
                    ph = pse.tile([128, HID], f32)
                    nc.tensor.matmul(ph, lhsT=pss1.pop(t), rhs=w1s,
                                     start=True, stop=True)
                    ht = tp2.tile([128, HID], bf16, tag="ht")
                    if nzbias:
                        tb = tp2.tile([128, HID], f32, tag="tb")
                        nc.vector.tensor_scalar(tb, b1rs, degs[:, t:t + 1],
                                                None, AOP.mult)
                        hsum = tp2.tile([128, HID], f32, tag="hsum")
                        nc.vector.tensor_tensor(hsum, ph, tb, AOP.add)
                        nc.scalar.activation(ht, hsum, ACT.Relu)
                    else:
                        nc.scalar.activation(ht, ph, ACT.Relu)
                    nc.sync.dma_start(out=HS[t * 128:(t + 1) * 128, 0:HID],
                                      in_=ht)

            spmm_layer(XG, IN_DIM, its, epi1_tile, epi1_group, "a",
                       [cfg.GE] * (NT // cfg.GE)
                       + ([NT % cfg.GE] if NT % cfg.GE else []))
            if kdbg:
                nc.sync.dma_start(out=HDBG[:, :], in_=HS[:, 0:HID])
            if not timing:
                nc.gpsimd.collective_compute(
                    "AllGather", mybir.AluOpType.bypass,
                    replica_groups=[list(range(cfg.M))],
                    ins=[HS[:, :]], outs=[HF[:, :]])

        # ================= layer 2 + fused classifier/log_softmax
        with tc.tile_pool(name="te1", bufs=cfg.GE + 2) as te1, \
             tc.tile_pool(name="te2", bufs=3) as te2, \
             tc.tile_pool(name="te3", bufs=2) as te3, \
             tc.tile_pool(name="psf", bufs=3, space="PSUM") as psf:
            G = cfg.LNG
            _rem = NT - cfg.LNG * (NT // cfg.LNG - 1)
            gplan2 = [cfg.LNG] * (NT // cfg.LNG - 1) + [_rem - 5, 3, 2]
            gname = {}
            acc = 0
            for gi, gsz in enumerate(gplan2):
                for i in range(gsz):
                    gname[acc + i] = (gi, i, gsz)
                acc += gsz
            pss2 = {}
            state = {}

            def epi2_tile(t, ps):
                pss = te1.tile([HID, 128], bf16, tag="pss", name="pss")
                nc.scalar.activation(pss, ps, ACT.Copy)
                pss2[t] = pss

            def epi2_group(t0, n):
                for t in range(t0, t0 + n):
                    g, i, gsz = gname[t]
                    if i == 0:
                        state["lgg"] = te3.tile([128, G, NCLS], f32,
                                                tag="lgg", name="lgg")
                        state["negg"] = te3.tile([128, G], f32,
                                                 tag="negg", name="negg")
                        state["smg"] = te3.tile([128, G], f32,
                                                tag="smg", name="smg")
                    lgg, negg, smg = state["lgg"], state["negg"], state["smg"]
                    psl = psf.tile([128, NCLS], f32)
                    nc.tensor.matmul(psl, lhsT=pss2.pop(t), rhs=w2cs,
                                     start=True, stop=True)
                    if nzbias:
                        tb = te2.tile([128, NCLS], f32, tag="tb")
                        nc.vector.tensor_scalar(tb, bcombs, degs[:, t:t + 1],
                                                None, AOP.mult)
                        lg0 = te2.tile([128, NCLS], f32, tag="lg0")
                        nc.vector.tensor_tensor(lg0, psl, tb, AOP.add)
                        nc.gpsimd.tensor_tensor(lgg[:, i, :], lg0, bcrs,
                                                AOP.add)
                    else:
                        nc.scalar.activation(lgg[:, i, :], psl, ACT.Copy)
                    if i == gsz - 1:
                        if kdbg:
                            nc.sync.dma_start(
                                out=LDBG[:, g * G * NCLS:(g + 1) * G * NCLS],
                                in_=lgg.rearrange("p a b -> p (a b)"))
                            nc.sync.dma_start(out=SMDBG[:, g * G:(g + 1) * G],
                                              in_=smg)
                        nc.vector.tensor_reduce(negg[:, 0:gsz],
                                                lgg[:, 0:gsz, :],
                                                mybir.AxisListType.X, AOP.max,
                                                negate=True)
                        for ii in range(gsz):
                            et = te2.tile([128, NCLS], f32, tag="et")
                            nc.scalar.activation(et, lgg[:, ii, :], ACT.Exp,
                                                 bias=negg[:, ii:ii + 1],
                                                 accum_out=smg[:, ii:ii + 1])
                        lng = te2.tile([128, G], f32, tag="lng")
                        nc.scalar.activation(lng[:, 0:gsz], smg[:, 0:gsz],
                                             ACT.Ln)
                        shg = te2.tile([128, G], f32, tag="shg")
                        nc.vector.tensor_tensor(shg[:, 0:gsz], negg[:, 0:gsz],
                                                lng[:, 0:gsz], AOP.subtract)
                        for ii in range(gsz):
                            tt = t - gsz + 1 + ii
                            ot = te2.tile([128, NCLS], f16, tag="ot")
                            nc.vector.tensor_scalar(ot, lgg[:, ii, :],
                                                    shg[:, ii:ii + 1], None,
                                                    AOP.add)
                            nc.sync.dma_start(
                                out=OUT[tt * 128:(tt + 1) * 128, :], in_=ot)

            spmm_layer(HF, HID, its, epi2_tile, epi2_group, "b", gplan2)

    nc.compile()
    return nc


_NC_CACHE = {}


def _get_nc(cfg):
    key = (cfg.NT, cfg.KSEG, cfg.SLABC, cfg.NZBIAS)
    if key not in _NC_CACHE:
        _NC_CACHE[key] = _build(cfg, nzbias=cfg.NZBIAS)
    return _NC_CACHE[key]


# ------------------------------------------------------------------ main ---
def kernel(x, edge_row, edge_col, edge_val, W1, b1, W2, b2, Wc, bc,
           _run_kwargs=None):
    from concourse.bass_utils import run_bass_kernel_spmd

    cfg = CFG
    x = np.asarray(x, dtype=np.float32)
    edge_row = np.asarray(edge_row, dtype=np.int64)
    edge_col = np.asarray(edge_col, dtype=np.int64)
    edge_val = np.asarray(edge_val, dtype=np.float32)
    W1 = np.asarray(W1, dtype=np.float32)
    W2 = np.asarray(W2, dtype=np.float32)
    Wc = np.asarray(Wc, dtype=np.float32)
    b1 = np.asarray(b1, dtype=np.float32)
    b2 = np.asarray(b2, dtype=np.float32)
    bc = np.asarray(bc, dtype=np.float32)

    cfg.NZBIAS = bool(np.any(b1) or np.any(b2) or np.any(bc))
    slot_of = _assign_slots(cfg, edge_row, edge_col)
    try:
        idx_all, ldst_all, val_all, deg_all = _plan(
            cfg, edge_row, edge_col, edge_val, slot_of)
    except ValueError:
        cfg.KSEG += 1
        idx_all, ldst_all, val_all, deg_all = _plan(
            cfg, edge_row, edge_col, edge_val, slot_of)

    xg = np.zeros((cfg.NPAD, cfg.IN_DIM), dtype=ml_dtypes.bfloat16)
    xg[slot_of] = x.astype(ml_dtypes.bfloat16)

    w1h = W1.astype(ml_dtypes.bfloat16)
    w2c = (W2 @ Wc).astype(ml_dtypes.bfloat16)
    bcomb = b2 @ Wc
    iota = np.tile(np.arange(128, dtype=np.float32), (128, 1)).astype(
        ml_dtypes.bfloat16)
    b1r = np.tile(b1, (128, 1)).astype(np.float32)
    bcombr = np.tile(bcomb, (128, 1)).astype(np.float32)
    bcr = np.tile(bc, (128, 1)).astype(np.float32)

    nc = _get_nc(cfg)
    in_maps = []
    for c in range(cfg.M):
        in_maps.append({
            "xg": xg, "idx": idx_all[c], "ldst": ldst_all[c],
            "val": val_all[c], "deg": deg_all[c], "w1": w1h, "w2c": w2c,
            "b1r": b1r, "bcombr": bcombr, "bcr": bcr, "iota": iota,
        })
    kw = dict(_run_kwargs or {})
    res = run_bass_kernel_spmd(nc, in_maps, core_ids=list(range(cfg.M)), **kw)
    shard = np.concatenate(
        [res.results[c]["out"] for c in range(cfg.M)], axis=0)  # [NPAD, NCLS]
    out = shard[slot_of]
    kernel.last_results = res
    return out.astype(np.float32)


# revision 46
# speedup vs baseline: 1.0249x; 1.0071x over previous
"""GCN node classifier (2x spmm + classifier + log_softmax) on 8 trn2 cores.

Strategy: destination-node 1D sharding with spmm linearity.
  spmm(A, x@W1 + b1) = (A x)@W1 + deg * b1^T      (deg = rowsum of A)
  spmm(A, h@W2 + b2)@Wc = (A h)@(W2 Wc) + deg * (b2 Wc)^T
so the gather tables are the RAW node features (x bf16 for layer 1,
relu-h bf16 for layer 2) — no dense pre-pass over all nodes, and the
layer weights are applied per dst tile after aggregation.

Each core owns 12,800 dst slots (100 tiles x 128 lanes). Host assigns
nodes to slots with a greedy 4-d balancer so that every (src-quarter,
dst-tile) edge bucket fits in KSEG=4 chunks of 128 edges (the int16
gather index forces 4 quarter views of the 102,400-row table). Per-edge
source rows are fetched with GPSIMD dma_gather (256B rows); the
segment-sum is a tensor-engine matmul against per-chunk scatter
matrices V[e, dst_lane] = edge_val[e] built on DVE with
(iota == ldst) * val, accumulated transposed (psT = Xg^T V) so the
per-tile epilogue can feed psT straight back as lhsT for the weight
matmul. log-softmax is fused per tile. Between layers the per-shard
relu-h table is AllGather'ed into a Shared DRAM tensor.
"""

import numpy as np
import ml_dtypes

from contextlib import ExitStack


# ---------------------------------------------------------------- config ---
class Cfg:
    M = 8                 # cores
    N_NODES = 100000
    N_EDGES = 1600000
    IN_DIM = 128
    HID = 64
    NCLS = 40
    NT = 99               # dst tiles per core (128 lanes each)
    KSEG = 4              # chunks (of 128 edges) per (quarter, tile) segment
    SLABC = 9             # chunks per gather slab
    SINGLE_PACKET = False  # multi-packet gathers (single-packet hangs >~1K idxs)
    NQUEUES = 4           # spread gathers over all 4 SWDGE queues
    MSGBUFS = 30
    IDXBUFS = 1
    GE = 5                # tiles per epilogue-matmul batch
    POOLV = 1             # of 16 chunks/tile, how many V-builds go to Pool
    PSB = 3               # psum accumulator ring depth
    VPB = 8               # V-tile ring depth
    LNG = 5               # tiles per deferred-Ln group
    NZBIAS = False        # set per-input: any of b1/b2/bc nonzero

    @property
    def PADSHARD(self):
        return self.NT * 128

    @property
    def NPAD(self):
        return self.PADSHARD * self.M

    @property
    def QROWS(self):
        return self.NPAD // 4

    @property
    def SEG(self):
        return self.KSEG * 128

    @property
    def CQ(self):
        return self.NT * self.KSEG          # chunks per quarter

    @property
    def NSLAB(self):
        assert self.CQ % self.SLABC == 0
        return self.CQ // self.SLABC        # gather slabs per quarter

    @property
    def CHUNKS(self):
        return 4 * self.CQ


CFG = Cfg()


# ------------------------------------------------------------- host plan ---
def _assign_slots(cfg, edge_row, edge_col):
    """Assign nodes to table slots so every (src-quarter, dst-tile) edge
    bucket holds <= KSEG*128 edges. Returns slot_of[node] -> [0, NPAD).

    Nodes are first split into 4 fixed quarter groups (so each node's
    src-quarter is pinned), then greedily packed into the 2*NT tiles of
    their own quarter balancing the 4-vector of per-src-quarter in-edge
    counts.
    """
    N, NPAD, QROWS, NT, M = cfg.N_NODES, cfg.NPAD, cfg.QROWS, cfg.NT, cfg.M
    TPQ = QROWS // 128                       # tiles per quarter (2 cores)
    rng = np.random.default_rng(12345)
    order = rng.permutation(N)
    qgrp = np.empty(N, dtype=np.int64)       # node -> quarter group
    npq = N // 4
    for q in range(4):
        qgrp[order[q * npq:(q + 1) * npq]] = q
    qgrp[order[4 * npq:]] = 3

    # per-node in-edge count by source quarter
    cnt = np.zeros((N, 4), dtype=np.int64)
    np.add.at(cnt, (edge_row, qgrp[edge_col]), 1)

    slot_of = np.empty(N, dtype=np.int64)
    for q in range(4):
        nodes = np.where(qgrp == q)[0]
        c = cnt[nodes].astype(np.float32)            # [nq, 4]
        tot = c.sum(1)
        o = np.argsort(-tot, kind="stable")
        nodes, c = nodes[o], c[o]
        loads = np.zeros((TPQ, 4), dtype=np.float32)
        fill = np.zeros(TPQ, dtype=np.int64)
        pos = np.empty(nodes.size, dtype=np.int64)
        for i in range(nodes.size):
            cand = np.max(loads + c[i], axis=1) + (fill >= 128) * 1e9
            b = int(np.argmin(cand))
            loads[b] += c[i]
            pos[i] = b * 128 + fill[b]
            fill[b] += 1
        slot_of[nodes] = q * QROWS + pos
    return slot_of


def _plan(cfg, edge_row, edge_col, edge_val, slot_of):
    """Bucket/sort/pad edges per core. Returns per-core arrays:
    idx16 [128, 4*CQ*128/16] int16, ldstT/valT [128, CHUNKS] bf16,
    plus degs [128, NT] f32 per core.
    """
    M, NT, KSEG, SEG, CQ, QROWS = cfg.M, cfg.NT, cfg.KSEG, cfg.SEG, cfg.CQ, cfg.QROWS
    PADSHARD = cfg.PADSHARD

    src_slot = slot_of[edge_col]
    dst_slot = slot_of[edge_row]
    q_of = src_slot // QROWS
    i_of = src_slot % QROWS
    core_of = dst_slot // PADSHARD
    dloc = dst_slot % PADSHARD
    t_of = dloc // 128
    l_of = dloc % 128

    deg = np.zeros(cfg.NPAD, dtype=np.float64)
    np.add.at(deg, dst_slot, edge_val.astype(np.float64))

    L = 4 * CQ * 128
    idx_all, ldst_all, val_all, deg_all = [], [], [], []
    for c in range(M):
        sel = core_of == c
        segid = q_of[sel] * NT + t_of[sel]
        order = np.argsort(segid, kind="stable")
        sid = segid[order]
        idx_s = i_of[sel][order]
        l_s = l_of[sel][order]
        v_s = edge_val[sel][order]

        counts = np.bincount(sid, minlength=4 * NT)
        if counts.max() > SEG:
            raise ValueError(f"segment overflow: {counts.max()} > {SEG}")
        starts = np.arange(4 * NT) * SEG
        pos = starts[sid] + (np.arange(sid.size)
                             - np.concatenate(([0], np.cumsum(counts)))[sid])

        idx = np.zeros(L, dtype=np.int16)
        ldst = np.zeros(L, dtype=np.float32)
        val = np.zeros(L, dtype=np.float32)
        idx[pos] = idx_s.astype(np.int16)
        ldst[pos] = l_s.astype(np.float32)
        val[pos] = v_s.astype(np.float32)

        # wrap indices: idx i -> [i%16, i//16], replicated on all 8 q7 cores
        idxw = np.tile(idx.reshape(-1, 16).T, (8, 1)).copy()
        ldstT = np.ascontiguousarray(ldst.reshape(-1, 128).T)
        valT = np.ascontiguousarray(val.reshape(-1, 128).T)
        degs = np.ascontiguousarray(
            deg[c * PADSHARD:(c + 1) * PADSHARD].reshape(NT, 128).T
        ).astype(np.float32)
        idx_all.append(idxw)
        ldst_all.append(ldstT)
        val_all.append(valT)
        deg_all.append(degs)
    return idx_all, ldst_all, val_all, deg_all


# --------------------------------------------------------- device program ---
def _build(cfg, timing=False, nzbias=False):
    import os
    from concourse import bacc, tile
    import concourse.mybir as mybir
    kdbg = bool(os.environ.get("KDBG"))

    f32 = mybir.dt.float32
    bf16 = mybir.dt.bfloat16
    i16 = mybir.dt.int16
    AOP = mybir.AluOpType
    ACT = mybir.ActivationFunctionType

    nc = bacc.Bacc("TRN2", target_bir_lowering=False, debug=False,
                   num_devices=1 if timing else cfg.M,
                   dynamic_dma_scratch_size=16384,
                   num_swdge_queues=cfg.NQUEUES)

    NPAD, QROWS, NT, KSEG, CQ, SLABC, NSLAB = (
        cfg.NPAD, cfg.QROWS, cfg.NT, cfg.KSEG, cfg.CQ, cfg.SLABC, cfg.NSLAB)
    CHUNKS, HID, NCLS, IN_DIM = cfg.CHUNKS, cfg.HID, cfg.NCLS, cfg.IN_DIM
    LQ16 = CQ * 128 // 16              # idx columns per quarter
    SLAB16 = SLABC * 128 // 16         # idx columns per slab

    # -------- I/O
    XG = nc.dram_tensor("xg", [NPAD, IN_DIM], bf16, kind="ExternalInput")
    IDX = nc.dram_tensor("idx", [128, 4 * LQ16], i16, kind="ExternalInput")
    LDST = nc.dram_tensor("ldst", [128, CHUNKS], f32, kind="ExternalInput")
    VAL = nc.dram_tensor("val", [128, CHUNKS], f32, kind="ExternalInput")
    DEG = nc.dram_tensor("deg", [128, NT], f32, kind="ExternalInput")
    W1 = nc.dram_tensor("w1", [IN_DIM, HID], bf16, kind="ExternalInput")
    W2C = nc.dram_tensor("w2c", [HID, NCLS], bf16, kind="ExternalInput")
    B1R = nc.dram_tensor("b1r", [128, HID], f32, kind="ExternalInput")
    BCOMBR = nc.dram_tensor("bcombr", [128, NCLS], f32, kind="ExternalInput")
    BCR = nc.dram_tensor("bcr", [128, NCLS], f32, kind="ExternalInput")
    IOTA = nc.dram_tensor("iota", [128, 128], bf16, kind="ExternalInput")
    f16 = mybir.dt.float16
    OUT = nc.dram_tensor("out", [cfg.PADSHARD, NCLS], f16, kind="ExternalOutput")
    HDBG = (nc.dram_tensor("hdbg", [cfg.PADSHARD, HID], bf16,
                           kind="ExternalOutput") if kdbg else None)
    LDBG = (nc.dram_tensor("ldbg", [128, NT * NCLS], f32,
                           kind="ExternalOutput") if kdbg else None)
    SMDBG = (nc.dram_tensor("smdbg", [128, NT], f32,
                            kind="ExternalOutput") if kdbg else None)

    # -------- internal DRAM
    HS = nc.dram_tensor("hshard", [cfg.PADSHARD, 128], bf16)    # cols 64+: junk
    HF = nc.dram_tensor("hfull", [NPAD, 128], bf16, addr_space="Shared")

    with tile.TileContext(nc) as tc, ExitStack() as top:
        # idx quarter 0 + V-build operands first: the first gathers and
        # V-builds depend only on these, so they issue before the consts.
        # one idx tile set serves BOTH layers (identical edge plan)
        idxp = top.enter_context(tc.tile_pool(name="idxp", bufs=1))
        its = [None] * 4
        its[0] = idxp.tile([128, LQ16], i16, tag="idx_0", name="idx_0")
        nc.sync.dma_start(out=its[0], in_=IDX[:, 0:LQ16])

        cpool = top.enter_context(tc.tile_pool(name="consts", bufs=1))
        iot = cpool.tile([128, 128], bf16)
        nc.sync.dma_start(out=iot, in_=IOTA[:, :])

        edg = top.enter_context(tc.tile_pool(name="edg", bufs=1))
        ldsts = edg.tile([128, CHUNKS], f32)
        nc.sync.dma_start(out=ldsts, in_=LDST[:, :])
        vals = edg.tile([128, CHUNKS], f32)
        nc.sync.dma_start(out=vals, in_=VAL[:, :])

        for q in range(1, 4):
            its[q] = idxp.tile([128, LQ16], i16, tag=f"idx_{q}",
                               name=f"idx_{q}")
            nc.sync.dma_start(out=its[q],
                              in_=IDX[:, q * LQ16:(q + 1) * LQ16])

        w1s = cpool.tile([IN_DIM, HID], bf16)
        nc.sync.dma_start(out=w1s, in_=W1[:, :])
        w2cs = cpool.tile([HID, NCLS], bf16)
        nc.sync.dma_start(out=w2cs, in_=W2C[:, :])
        b1rs = bcombs = bcrs = degs = None
        if nzbias:
            b1rs = cpool.tile([128, HID], f32)
            nc.sync.dma_start(out=b1rs, in_=B1R[:, :])
            bcombs = cpool.tile([128, NCLS], f32)
            nc.sync.dma_start(out=bcombs, in_=BCOMBR[:, :])
            bcrs = cpool.tile([128, NCLS], f32)
            nc.sync.dma_start(out=bcrs, in_=BCR[:, :])
            degs = cpool.tile([128, NT], f32)
            nc.sync.dma_start(out=degs, in_=DEG[:, :])


        # ============ spmm layer runner: per-tile single psum group across
        # all 4 quarters, accumulating transposed (psT = Xg^T V); epilogue
        # split into a per-tile part (cast) and a batched per-GE-tiles part
        # (weight matmuls etc) to keep the PE stream free of cross-engine
        # round trips.
        msg = top.enter_context(tc.tile_pool(name="msg", bufs=cfg.MSGBUFS))
        vp = top.enter_context(tc.tile_pool(name="vp", bufs=cfg.VPB))
        psb = top.enter_context(
            tc.tile_pool(name="psb", bufs=cfg.PSB, space="PSUM"))

        def spmm_layer(tab, width, lits, epi_tile, epi_group, ltag, gplan):
            gend = {}
            acc = 0
            for gsz in gplan:
                acc += gsz
                gend[acc - 1] = gsz
            assert acc == NT
            slabs = [[None] * NSLAB for _ in range(4)]

            def ensure_slab(q, s):
                if slabs[q][s] is None:
                    mt = msg.tile([128, SLABC, 128], bf16)
                    nc.gpsimd.dma_gather(
                        mt, tab[q * QROWS:(q + 1) * QROWS, :],
                        lits[q][:, s * SLAB16:(s + 1) * SLAB16],
                        num_idxs=SLABC * 128, num_idxs_reg=SLABC * 128,
                        elem_size=128, elem_step=128,
                        single_packet=cfg.SINGLE_PACKET,
                        queue_num=(q * NSLAB + s) % cfg.NQUEUES)
                    slabs[q][s] = mt
                return slabs[q][s]

            for t in range(NT):
                psfull = psb.tile([128, 128], f32, tag="ps", name="ps")
                ps = psfull if width == 128 else psfull[0:width, :]
                for q in range(4):
                    j0 = t * KSEG
                    vt = vp.tile([128, KSEG, 128], bf16)
                    for k in range(KSEG):
                        gj = q * CQ + j0 + k             # global chunk
                        veng = (nc.gpsimd
                                if q * KSEG + k >= 16 - cfg.POOLV and t >= 8
                                else nc.vector)
                        veng.tensor_scalar(
                            vt[:, k, :], iot, ldsts[:, gj:gj + 1],
                            vals[:, gj:gj + 1], AOP.is_equal, AOP.mult)
                    for k in range(KSEG):
                        j = j0 + k                       # chunk in quarter
                        mt = ensure_slab(q, j // SLABC)
                        nc.tensor.matmul(ps, lhsT=mt[:, j % SLABC, 0:width],
                                         rhs=vt[:, k, :],
                                         start=(q == 0 and k == 0),
                                         stop=(q == 3 and k == KSEG - 1))
                epi_tile(t, ps)
                if t in gend:
                    epi_group(t - gend[t] + 1, gend[t])

        # ================= layer 1: h = relu((A x)@W1 + deg*b1^T), store bf16
        with tc.tile_pool(name="tc1", bufs=cfg.GE + 2) as tp1, \
             tc.tile_pool(name="tc2", bufs=3) as tp2, \
             tc.tile_pool(name="pse", bufs=3, space="PSUM") as pse:
            pss1 = {}

            def epi1_tile(t, ps):
                pss = tp1.tile([IN_DIM, 128], bf16, tag="pss", name="pss")
                nc.scalar.activation(pss, ps, ACT.Copy)
                pss1[t] = pss

            def epi1_group(t0, n):
                for t in range(t0, t0 + n):
                    ph = pse.tile([128, HID], f32)
                    nc.tensor.matmul(ph, lhsT=pss1.pop(t), rhs=w1s,
                                     start=True, stop=True)
                    ht = tp2.tile([128, HID], bf16, tag="ht")
                    if nzbias:
                        tb = tp2.tile([128, HID], f32, tag="tb")
                        nc.vector.tensor_scalar(tb, b1rs, degs[:, t:t + 1],
                                                None, AOP.mult)
                        hsum = tp2.tile([128, HID], f32, tag="hsum")
                        nc.vector.tensor_tensor(hsum, ph, tb, AOP.add)
                        nc.scalar.activation(ht, hsum, ACT.Relu)
                    else:
                        nc.scalar.activation(ht, ph, ACT.Relu)
                    nc.sync.dma_start(out=HS[t * 128:(t + 1) * 128, 0:HID],
                                      in_=ht)

            spmm_layer(XG, IN_DIM, its, epi1_tile, epi1_group, "a",
                       [cfg.GE] * (NT // cfg.GE)
                       + ([NT % cfg.GE] if NT % cfg.GE else []))
            if kdbg:
                nc.sync.dma_start(out=HDBG[:, :], in_=HS[:, 0:HID])
            if not timing:
                nc.gpsimd.collective_compute(
                    "AllGather", mybir.AluOpType.bypass,
                    replica_groups=[list(range(cfg.M))],
                    ins=[HS[:, :]], outs=[HF[:, :]])

        # ================= layer 2 + fused classifier/log_softmax
        with tc.tile_pool(name="te1", bufs=cfg.GE + 2) as te1, \
             tc.tile_pool(name="te2", bufs=3) as te2, \
             tc.tile_pool(name="te3", bufs=2) as te3, \
             tc.tile_pool(name="psf", bufs=3, space="PSUM") as psf:
            G = cfg.LNG
            _rem = NT - cfg.LNG * (NT // cfg.LNG - 1)
            gplan2 = [cfg.LNG] * (NT // cfg.LNG - 1) + [_rem - 5, 3, 2]
            gname = {}
            acc = 0
            for gi, gsz in enumerate(gplan2):
                for i in range(gsz):
                    gname[acc + i] = (gi, i, gsz)
                acc += gsz
            pss2 = {}
            state = {}

            def epi2_tile(t, ps):
                pss = te1.tile([HID, 128], bf16, tag="pss", name="pss")
                nc.scalar.activation(pss, ps, ACT.Copy)
                pss2[t] = pss

            def epi2_group(t0, n):
                for t in range(t0, t0 + n):
                    g, i, gsz = gname[t]
                    if i == 0:
                        state["lgg"] = te3.tile([128, G, NCLS], f32,
                                                tag="lgg", name="lgg")
                        state["negg"] = te3.tile([128, G], f32,
                                                 tag="negg", name="negg")
                        state["smg"] = te3.tile([128, G], f32,
                                                tag="smg", name="smg")
                    lgg, negg, smg = state["lgg"], state["negg"], state["smg"]
                    psl = psf.tile([128, NCLS], f32)
                    nc.tensor.matmul(psl, lhsT=pss2.pop(t), rhs=w2cs,
                                     start=True, stop=True)
                    if nzbias:
                        tb = te2.tile([128, NCLS], f32, tag="tb")
                        nc.vector.tensor_scalar(tb, bcombs, degs[:, t:t + 1],
                                                None, AOP.mult)
                        lg0 = te2.tile([128, NCLS], f32, tag="lg0")
                        nc.vector.tensor_tensor(lg0, psl, tb, AOP.add)
                        nc.gpsimd.tensor_tensor(lgg[:, i, :], lg0, bcrs,
                                                AOP.add)
                    else:
                        nc.scalar.activation(lgg[:, i, :], psl, ACT.Copy)
                    if i == gsz - 1:
                        if kdbg:
                            nc.sync.dma_start(
                                out=LDBG[:, g * G * NCLS:(g + 1) * G * NCLS],
                                in_=lgg.rearrange("p a b -> p (a b)"))
                            nc.sync.dma_start(out=SMDBG[:, g * G:(g + 1) * G],
                                              in_=smg)
                        nc.vector.tensor_reduce(negg[:, 0:gsz],
                                                lgg[:, 0:gsz, :],
                                                mybir.AxisListType.X, AOP.max,
                                                negate=True)
                        for ii in range(gsz):
                            et = te2.tile([128, NCLS], f32, tag="et")
                            nc.scalar.activation(et, lgg[:, ii, :], ACT.Exp,
                                                 bias=negg[:, ii:ii + 1],
                                                 accum_out=smg[:, ii:ii + 1])
                        lng = te2.tile([128, G], f32, tag="lng")
                        nc.scalar.activation(lng[:, 0:gsz], smg[:, 0:gsz],
                                             ACT.Ln)
                        shg = te2.tile([128, G], f32, tag="shg")
                        nc.vector.tensor_tensor(shg[:, 0:gsz], negg[:, 0:gsz],
                                                lng[:, 0:gsz], AOP.subtract)
                        for ii in range(gsz):
                            tt = t - gsz + 1 + ii
                            ot = te2.tile([128, NCLS], f16, tag="ot")
                            nc.vector.tensor_scalar(ot, lgg[:, ii, :],
                                                    shg[:, ii:ii + 1], None,
                                                    AOP.add)
                            nc.sync.dma_start(
                                out=OUT[tt * 128:(tt + 1) * 128, :], in_=ot)

            spmm_layer(HF, HID, its, epi2_tile, epi2_group, "b", gplan2)

    nc.compile()
    return nc


_NC_CACHE = {}


def _get_nc(cfg):
    key = (cfg.NT, cfg.KSEG, cfg.SLABC, cfg.NZBIAS)
    if key not in _NC_CACHE:
        _NC_CACHE[key] = _build(cfg, nzbias=cfg.NZBIAS)
    return _NC_CACHE[key]


# ------------------------------------------------------------------ main ---
def kernel(x, edge_row, edge_col, edge_val, W1, b1, W2, b2, Wc, bc,
           _run_kwargs=None):
    from concourse.bass_utils import run_bass_kernel_spmd

    cfg = CFG
    x = np.asarray(x, dtype=np.float32)
    edge_row = np.asarray(edge_row, dtype=np.int64)
    edge_col = np.asarray(edge_col, dtype=np.int64)
    edge_val = np.asarray(edge_val, dtype=np.float32)
    W1 = np.asarray(W1, dtype=np.float32)
    W2 = np.asarray(W2, dtype=np.float32)
    Wc = np.asarray(Wc, dtype=np.float32)
    b1 = np.asarray(b1, dtype=np.float32)
    b2 = np.asarray(b2, dtype=np.float32)
    bc = np.asarray(bc, dtype=np.float32)

    cfg.NZBIAS = bool(np.any(b1) or np.any(b2) or np.any(bc))
    slot_of = _assign_slots(cfg, edge_row, edge_col)
    try:
        idx_all, ldst_all, val_all, deg_all = _plan(
            cfg, edge_row, edge_col, edge_val, slot_of)
    except ValueError:
        cfg.KSEG += 1
        idx_all, ldst_all, val_all, deg_all = _plan(
            cfg, edge_row, edge_col, edge_val, slot_of)

    xg = np.zeros((cfg.NPAD, cfg.IN_DIM), dtype=ml_dtypes.bfloat16)
    xg[slot_of] = x.astype(ml_dtypes.bfloat16)

    w1h = W1.astype(ml_dtypes.bfloat16)
    w2c = (W2 @ Wc).astype(ml_dtypes.bfloat16)
    bcomb = b2 @ Wc
    iota = np.tile(np.arange(128, dtype=np.float32), (128, 1)).astype(
        ml_dtypes.bfloat16)
    b1r = np.tile(b1, (128, 1)).astype(np.float32)
    bcombr = np.tile(bcomb, (128, 1)).astype(np.float32)
    bcr = np.tile(bc, (128, 1)).astype(np.float32)

    nc = _get_nc(cfg)
    in_maps = []
    for c in range(cfg.M):
        in_maps.append({
            "xg": xg, "idx": idx_all[c], "ldst": ldst_all[c],
            "val": val_all[c], "deg": deg_all[c], "w1": w1h, "w2c": w2c,
            "b1r": b1r, "bcombr": bcombr, "bcr": bcr, "iota": iota,
        })
    kw = dict(_run_kwargs or {})
    res = run_bass_kernel_spmd(nc, in_maps, core_ids=list(range(cfg.M)), **kw)
    shard = np.concatenate(
        [res.results[c]["out"] for c in range(cfg.M)], axis=0)  # [NPAD, NCLS]
    out = shard[slot_of]
    kernel.last_results = res
    return out.astype(np.float32)
